# revision 45
# baseline (speedup 1.0000x reference)
"""MicroTransformer (B=16,S=512,V=8000,D=5,F=20,L=2) on 8 trn2 NeuronCores.

Sharding: pure data parallel over batch (2 batch elements per core).
All parameters replicated. Whole transformer body + logits matmul run on
device; host only does input prep (embedding row gather, positional
encoding constant, weight layout transforms) and the final reshape.

Per-core device program (Bass/Tile, fully unrolled), v2:
  state h [6, 1024] f32r: rows 0-4 = h^T for batch0|batch1, row 5 = ones.
  qkv+vT in ONE matmul: stationary [6,69] (q scaled at cols 0-4, k at
  32-36, vT at 64-68 -> all psum reads 32-aligned).  V is transposed to
  [128k, 5] per chunk via 4 PE-transposes (identity stationary) and cast
  bf16 into the v-aug tile (ones col 32 accumulates Z).
  Attention transposed ([k,q]), softmax without row-max; exp -> bf16,
  causal mask = in-place [128,128] bf16 multiply on the diagonal block
  only (off-diagonal needs no mask).
  LayerNorm: stats via 2 matmuls into ONE psum bank (s1@row0, s2@row32,
  g*rstd@64, g*mu*rstd@96); rstd = Exp(-0.5*Ln(var+eps)) so every ACT
  function (Exp/Ln/Square/Copy/Relu-free) lives in ONE table set (no
  mid-kernel ACT table swaps).  Broadcast matmuls take a zero-initialized
  [33,512] moving tile with rstd@row0, mu*rstd@row32.
  Logits (the memory-bound phase): final h -> hfin [6,1024] bf16; fcw
  [6,8192] bf16 (vocab padded).  Flat 512-col matmuls, stationary = one
  [6,128] token chunk reused for 16 consecutive matmuls (no row-tiling,
  no fcw replicas).  Psum ring = 2x [128,1024] tiles; casts to fp16
  stage tiles ping-pong DVE/ACT; 0.5MB stores alternate the sync and
  gpsimd HWDGE rings so DMA streams continuously.  Batch 0's logits are
  emitted right after its final LN2 with batch 1's ffn tail interleaved
  mid-stream, so stores start as early as possible and never gap.
"""

import math

import numpy as np
import ml_dtypes

import concourse.bacc as bacc
import concourse.bass as bass
import concourse.mybir as mybir
import concourse.tile as tile
from concourse.bass_utils import run_bass_kernel_spmd

F32 = mybir.dt.float32
F32R = mybir.dt.float32r
BF16 = mybir.dt.bfloat16
F16 = mybir.dt.float16
ALU = mybir.AluOpType
ACTF = mybir.ActivationFunctionType


def _r(ap):
    return ap.bitcast(F32R)


def _f(ap):
    return ap.bitcast(F32)


B, S, V, D, F, L = 16, 512, 8000, 5, 20, 2
VP = 8192                      # vocab padded to 16x512
EPS = 1e-5
NCORES = 8
BPC = B // NCORES              # 2
SQRT_D = math.sqrt(float(D))
SCALE = 1.0 / SQRT_D
QC = S // 128                  # 4
S2 = BPC * S                   # 1024
NR = VP // 1024                # 8 logit rounds per (b, i)

_CACHED = {}

_ACT_SET = "natural_log_exp_and_others"


def _pin_act_tables():
    """Expose only the one table set covering Exp/Ln/Square/Copy.

    The act-table placement pass picks the first set containing each
    activation's function; with the full list it ping-pongs between the
    exp set and the ln set on every LayerNorm (measured 17 mid-kernel
    ~1.3us table loads).  Restricting the choice to the single covering
    set yields exactly one load at kernel start.
    """
    orig = bacc.get_activation_tables

    def only_covering(arch):
        t = orig(arch)
        # act_func_set_id is positional into act_info.json, so keep every
        # entry in place and just make the non-covering sets unchoosable.
        return {name: (funcs if name == _ACT_SET else set())
                for name, funcs in t.items()}

    bacc.get_activation_tables = only_covering
    return orig


def _build_program(iters=1):
    _orig_tables = _pin_act_tables()
    try:
        return _build_program_inner(iters)
    finally:
        bacc.get_activation_tables = _orig_tables


def _build_program_inner(iters=1):
    nc = bacc.Bacc("TRN2", target_bir_lowering=False, debug=False,
                   num_devices=NCORES)

    d_h0 = nc.dram_tensor("h0", [D + 1, S2], F32R, kind="ExternalInput")
    d_pack = nc.dram_tensor("packw", [37, 288], F32R, kind="ExternalInput")
    d_mask = nc.dram_tensor("mask", [128, 128], F32, kind="ExternalInput")
    d_ctxa0 = nc.dram_tensor("ctxa0", [D + 1, S2], F32R,
                             kind="ExternalInput")
    d_f1a0 = nc.dram_tensor("f1a0", [F + 1, S2], F32R, kind="ExternalInput")
    d_vsb0 = nc.dram_tensor("vsb0", [128, QC, 37], F32R,
                            kind="ExternalInput")
    d_hf0 = nc.dram_tensor("hf0", [D + 1, S2], BF16, kind="ExternalInput")
    d_rr0 = nc.dram_tensor("rr0", [33, S], F32R, kind="ExternalInput")
    d_fcw = nc.dram_tensor("fcw", [D + 1, VP], BF16, kind="ExternalInput")
    d_out = nc.dram_tensor("out", [128, BPC, QC, VP], F16,
                           kind="ExternalOutput")

    from contextlib import ExitStack
    with tile.TileContext(nc) as tc, ExitStack() as es, \
            nc.allow_low_precision(reason="bf16/f16 rounding intended"):
        cst = es.enter_context(tc.tile_pool(name="cst", bufs=1))
        wrk = es.enter_context(tc.tile_pool(name="wrk", bufs=3))
        att = es.enter_context(tc.tile_pool(name="att", bufs=3))
        stg = es.enter_context(tc.tile_pool(name="stg", bufs=6))
        # one 4-bank pool set per batch: the two body chains never share
        # psum, so they truly run in parallel; each batch's logits phase
        # then recycles pools that are idle by that point.
        ps_cz = [es.enter_context(tc.tile_pool(name=f"ps_cz{b}", bufs=1,
                                               space="PSUM"))
                 for b in range(BPC)]
        ps_ms = [es.enter_context(tc.tile_pool(name=f"ps_ms{b}", bufs=3,
                                               space="PSUM"))
                 for b in range(BPC)]

        # ---- constants ----
        h = cst.tile([D + 1, S2], F32R, name="h", tag="h")
        nc.sync.dma_start(h[:], d_h0[:])
        pack = cst.tile([37, 288], F32R, name="pack", tag="pack")
        nc.sync.dma_start(pack[:], d_pack[:])

        wqkv = [pack[0:D + 1, 70 * l:70 * l + 69] for l in range(L)]
        wo = [pack[0:D + 1, 140 + D * l:140 + D * (l + 1)] for l in range(L)]
        w1 = [pack[0:D + 1, 150 + F * l:150 + F * (l + 1)] for l in range(L)]
        w2 = [pack[0:F + 1, 190 + D * l:190 + D * (l + 1)] for l in range(L)]
        grow0 = [pack[0:33, 200 + D * k:200 + D * (k + 1)] for k in range(4)]
        grow32 = [pack[0:33, 220 + D * k:220 + D * (k + 1)] for k in range(4)]
        c02 = pack[0:D, 240:241]       # stats stationary (1/D column)
        eye5 = _f(pack[0:D, 273:278])
        bcol = [_f(pack[0:D, 278 + k:279 + k]) for k in range(4)]
        ones5 = pack[0:1, 283:288]

        maskb = cst.tile([128, 128], F32, name="maskb", tag="maskb")
        nc.gpsimd.dma_start(maskb[:], d_mask[:])
        c30 = cst.tile([128, 1], F32, name="c30", tag="c30")
        nc.vector.memset(c30[:], -30.0)
        ctxa = cst.tile([D + 1, S2], F32R, name="ctxa", tag="ctxa")
        nc.gpsimd.dma_start(ctxa[:], d_ctxa0[:])
        f1a = cst.tile([F + 1, S2], F32R, name="f1a", tag="f1a")
        nc.gpsimd.dma_start(f1a[:], d_f1a0[:])
        vsb = []
        for b in range(BPC):
            t = cst.tile([128, QC, 37], F32R, name=f"vsb{b}", tag=f"vsb{b}")
            nc.gpsimd.dma_start(t[:], d_vsb0[:])
            vsb.append(t)
        rr = []
        for b in range(BPC):
            t = cst.tile([33, S], F32R, name=f"rr{b}", tag=f"rr{b}")
            nc.gpsimd.dma_start(t[:], d_rr0[:])
            rr.append(t)
        hfin = cst.tile([D + 1, S2], BF16, name="hfin", tag="hfin")
        nc.gpsimd.dma_start(hfin[:], d_hf0[:])
        fcw = cst.tile([D + 1, VP], BF16, name="fcw", tag="fcw")
        nc.gpsimd.dma_start(fcw[:], d_fcw[:])

        def layernorm(l, i, b, it, resid_ap, add_ps, out_ap):
            """out = LN(resid + add_ps) * g + b.

            Two independent 256-token half-chains; halving the free size
            halves every op's duration, and the halves pipeline across
            engines, roughly halving the ~13-step serial latency.
            """
            u = f"{l}{i}{b}_{it}"
            k = 2 * l + i
            HS = S // 2
            x = wrk.tile([D, S], F32R, name=f"lx{u}", tag="lx")
            xsq = wrk.tile([D, S], F32R, name=f"lq{u}", tag="lq")
            s1 = ps_ms[b].tile([1, S], F32, name=f"s1{u}", tag="ps_ms")
            s2 = ps_ms[b].tile([1, S], F32, name=f"s2{u}", tag="ps_ms")
            t1 = wrk.tile([1, S], F32, name=f"t1{u}", tag="lt1")
            var = wrk.tile([1, S], F32, name=f"lv{u}", tag="lvar")
            lnv = wrk.tile([1, S], F32, name=f"ll{u}", tag="llnv")
            rbp = ps_ms[b].tile([D, S], F32, name=f"lr{u}", tag="ps_ms")
            rmp = ps_ms[b].tile([D, S], F32, name=f"lm{u}", tag="ps_ms")
            t2 = wrk.tile([D, S], F32, name=f"t2{u}", tag="lt2")
            hs = [slice(0, HS), slice(HS, S)]
            for hh in hs:
                nc.vector.tensor_add(x[:, hh], resid_ap[:, hh],
                                     add_ps[:, hh])
            for hh in hs:
                nc.gpsimd.tensor_mul(xsq[:, hh], x[:, hh], x[:, hh])
                nc.tensor.matmul(s1[:, hh], c02, x[:, hh])
            for hh in hs:
                nc.tensor.matmul(s2[:, hh], c02, xsq[:, hh])
                nc.scalar.square(t1[:, hh], s1[:, hh])
            for hh in hs:
                nc.vector.scalar_tensor_tensor(var[:, hh], s2[:, hh], EPS,
                                               t1[:, hh],
                                               op0=ALU.add, op1=ALU.subtract)
            for hh in hs:
                nc.scalar.activation(lnv[:, hh], var[:, hh], ACTF.Ln)
            for hh in hs:
                nc.scalar.activation(rr[b][0:1, hh], lnv[:, hh], ACTF.Exp,
                                     scale=-0.5)
                nc.vector.tensor_mul(rr[b][32:33, hh], _f(rr[b][0:1, hh]),
                                     s1[:, hh])
            for hh in hs:
                nc.tensor.matmul(rbp[:, hh], grow0[k], rr[b][:, hh])
                nc.tensor.matmul(rmp[:, hh], grow32[k], rr[b][:, hh])
            for hh in hs:
                nc.vector.tensor_mul(t2[:, hh], _f(x[:, hh]), rbp[:, hh])
            for hh in hs:
                nc.vector.scalar_tensor_tensor(out_ap[:, hh], t2[:, hh],
                                               bcol[k], rmp[:, hh],
                                               op0=ALU.add, op1=ALU.subtract)

        def attn(l, b, it):
            u = f"{l}{b}_{it}"
            hb = h[:, b * S:(b + 1) * S]
            qkp = ps_ms[b].tile([69, S], F32, name=f"qk{u}", tag="ps_ms")
            nc.tensor.matmul(qkp[:], wqkv[l], hb)
            qsb = wrk.tile([D, S], F32R, name=f"qs{u}", tag="qsb")
            ksb = wrk.tile([D, S], F32R, name=f"ks{u}", tag="ksb")
            vTs = wrk.tile([D, S], F32, name=f"vt{u}", tag="vts")
            nc.scalar.copy(qsb[:], qkp[0:D, :])
            nc.vector.tensor_copy(ksb[:], qkp[32:32 + D, :])
            nc.scalar.copy(vTs[:], qkp[64:64 + D, :])
            # v -> [128k, 5] per chunk via PE transpose; cast bf16 into v-aug
            # (8-wide chunk stride keeps each transpose out 32B-aligned)
            vtp = ps_ms[b].tile([128, QC, 8], F32, name=f"vp{u}", tag="ps_ms")
            for kc in range(QC):
                nc.tensor.transpose(vtp[:, kc, 0:D],
                                    vTs[:, kc * 128:(kc + 1) * 128], eye5)
            nc.vector.tensor_copy(vsb[b][:, :, 0:D], vtp[:, :, 0:D])

            cz = ps_cz[b].tile([37, S], F32, name=f"cz{u}", tag="ps_cz")
            for kc in range(QC):
                off = kc * 128
                n = S - off
                scp = ps_ms[b].tile([128, S], F32, name=f"sc{u}{kc}",
                                    tag="ps_ms")
                nc.tensor.matmul(scp[:, 0:n], ksb[:, off:off + 128],
                                 qsb[:, off:S])
                expT = att.tile([128, S], F32R, name=f"ex{u}{kc}", tag="expT")
                # softmax is shift-invariant: exp(s-30) keeps the table's
                # input/output range comfortable (scores reach ~|50|)
                nc.scalar.activation(expT[:, 0:n], scp[:, 0:n], ACTF.Exp,
                                     bias=c30[:])
                nc.gpsimd.tensor_mul(expT[:, 0:128], expT[:, 0:128],
                                     maskb[:])
                nc.tensor.matmul(cz[:, off:S], vsb[b][:, kc, :],
                                 expT[:, 0:n],
                                 start=(kc == 0), stop=(kc == QC - 1))
            zf = wrk.tile([D, S], F32, name=f"zf{u}", tag="zf")
            nc.scalar.copy(zf[:], cz[32:32 + D, :])
            ctxs = wrk.tile([D, S], F32, name=f"cs{u}", tag="ctxs")
            nc.scalar.copy(ctxs[:], cz[0:D, :])
            zr = wrk.tile([D, S], F32, name=f"zr{u}", tag="zr")
            nc.vector.reciprocal_approx_fast(zr[:], zf[:])
            nc.gpsimd.tensor_mul(ctxa[0:D, b * S:(b + 1) * S], ctxs[:],
                                 zr[:])

        def proj_ln1(l, b, it):
            bc0 = b * S
            pp = ps_ms[b].tile([D, S], F32, name=f"pp{l}{b}_{it}", tag="ps_ms")
            nc.tensor.matmul(pp[:], wo[l], ctxa[:, bc0:bc0 + S])
            layernorm(l, 0, b, it, h[0:D, bc0:bc0 + S], pp[:],
                      h[0:D, bc0:bc0 + S])

        def ffn_ln2(l, b, it):
            bc0 = b * S
            f1p = ps_ms[b].tile([F, S], F32, name=f"f1{l}{b}_{it}", tag="ps_ms")
            nc.tensor.matmul(f1p[:], w1[l], h[:, bc0:bc0 + S])
            nc.vector.tensor_scalar_max(f1a[0:F, bc0:bc0 + S // 2],
                                        f1p[:, 0:S // 2], 0.0)
            nc.vector.tensor_scalar_max(f1a[0:F, bc0 + S // 2:bc0 + S],
                                        f1p[:, S // 2:S], 0.0)
            f2p = ps_ms[b].tile([D, S], F32, name=f"f2{l}{b}_{it}", tag="ps_ms")
            nc.tensor.matmul(f2p[:], w2[l], f1a[:, bc0:bc0 + S])
            if l == L - 1:
                layernorm(l, 1, b, it, h[0:D, bc0:bc0 + S], f2p[:],
                          hfin[0:D, bc0:bc0 + S])
            else:
                layernorm(l, 1, b, it, h[0:D, bc0:bc0 + S], f2p[:],
                          h[0:D, bc0:bc0 + S])

        def logits_b(b, it, tails=None):
            """32 rounds of [2 matmuls -> cast -> staged 0.5MB store].

            tails: {round_index: callback} -- emits the other batch's body
            segments between rounds so its work drains during this batch's
            store stream (engine queues are in-order; segments must be
            small enough not to starve the cast/store pipeline).
            """
            # this batch's own body pools are free once hfin is ready;
            # batch 1's logits can additionally recycle batch 0's pools
            # (its logits phase has completed by then) for a deeper ring.
            ring = [(ps_cz[b], "ps_cz"), (ps_ms[b], "ps_ms"),
                    (ps_ms[b], "ps_ms"), (ps_ms[b], "ps_ms")]
            if b == 1:
                ring = ring + [(ps_cz[0], "ps_cz"), (ps_ms[0], "ps_ms"),
                               (ps_ms[0], "ps_ms"), (ps_ms[0], "ps_ms")]
            nr = 0
            st = None
            for i in range(QC):
                stat = hfin[0:D + 1, b * S + 128 * i:b * S + 128 * (i + 1)]
                for r in range(VP // 512):
                    pool, tg = ring[nr % len(ring)]
                    lp = pool.tile([128, 512], F32,
                                   name=f"lp{b}{i}{r}_{it}", tag=tg)
                    v0 = r * 512
                    nc.tensor.matmul(lp[:], stat, fcw[:, v0:v0 + 512])
                    if r % 4 == 0:
                        st = stg.tile([128, 2048], F16,
                                      name=f"st{b}{i}{r}_{it}", tag="stage")
                    sl = st[:, (r % 4) * 512:(r % 4) * 512 + 512]
                    if nr % 2 == 0:
                        nc.vector.tensor_copy(sl, lp[:])
                    else:
                        nc.scalar.copy(sl, lp[:])
                    if r % 4 == 3:
                        eng = nc.sync if (nr // 4) % 2 == 0 else nc.gpsimd
                        eng.dma_start(d_out[:, b, i, v0 - 1536:v0 + 512],
                                      st[:])
                    nr += 1
                    if tails and nr in tails:
                        tails[nr]()

        for it in range(iters):
            if it > 0:
                nc.sync.dma_start(h[:], d_h0[:])
            # stepwise batch interleave: the two chains fill each other's
            # engine gaps, and both logits phases then run contention-free
            # (pure matmul+cast+store streams).
            for l in range(L):
                for b in range(BPC):
                    attn(l, b, it)
                for b in range(BPC):
                    proj_ln1(l, b, it)
                if l < L - 1:
                    for b in range(BPC):
                        ffn_ln2(l, b, it)
            # final-layer tail: emit batch 0's ffn+LN2, then its logits with
            # batch 1's ffn+LN2 injected mid-stream so batch 1's hfin is
            # ready just before its own logits rounds begin.
            ffn_ln2(L - 1, 0, it)
            logits_b(0, it,
                     tails={40: lambda it=it: ffn_ln2(L - 1, 1, it)})
            logits_b(1, it)

    nc.compile()
    return nc


def _get_program(iters=1):
    if iters not in _CACHED:
        _CACHED[iters] = _build_program(iters)
    return _CACHED[iters]


def _pos_encoding_np():
    pos = np.arange(B, dtype=np.float32)[:, None]
    div = np.exp(np.arange(0, D, 2, dtype=np.float32)
                 * (-math.log(10000.0) / D))
    pe = np.zeros((B, D), dtype=np.float32)
    pe[:, 0::2] = np.sin(pos * div)
    pe[:, 1::2] = np.cos(pos * div[:-1])
    return pe


def host_inputs(x, emb, in_proj_w, in_proj_b, out_proj_w, out_proj_b,
                ln1_g, ln1_b, ln2_g, ln2_b, ff1_w, ff1_b, ff2_w, ff2_b,
                fc_w, fc_b):
    x = np.asarray(x).astype(np.int64)
    emb = np.asarray(emb, dtype=np.float32)
    f32 = lambda a: np.ascontiguousarray(np.asarray(a, dtype=np.float32))
    in_proj_w, in_proj_b = f32(in_proj_w), f32(in_proj_b)
    out_proj_w, out_proj_b = f32(out_proj_w), f32(out_proj_b)
    ff1_w, ff1_b, ff2_w, ff2_b = f32(ff1_w), f32(ff1_b), f32(ff2_w), f32(ff2_b)
    ln1_g, ln1_b, ln2_g, ln2_b = f32(ln1_g), f32(ln1_b), f32(ln2_g), f32(ln2_b)
    fc_w, fc_b = f32(fc_w), f32(fc_b)

    h0 = emb[x] * np.float32(SQRT_D)
    h0 = h0 + _pos_encoding_np()[:, None, :]
    h0t = np.transpose(h0, (0, 2, 1))          # [B, D, S]

    def aug(wT, bias):
        return np.concatenate([wT, bias[None, :]], axis=0).astype(np.float32)

    packw = np.zeros((37, 288), np.float32)
    for l in range(L):
        packw[0:D + 1, 70 * l:70 * l + D] = aug(
            in_proj_w[l][0:D].T * SCALE, in_proj_b[l][0:D] * SCALE)
        packw[0:D + 1, 70 * l + 32:70 * l + 32 + D] = aug(
            in_proj_w[l][D:2 * D].T, in_proj_b[l][D:2 * D])
        packw[0:D + 1, 70 * l + 64:70 * l + 64 + D] = aug(
            in_proj_w[l][2 * D:3 * D].T, in_proj_b[l][2 * D:3 * D])
        packw[0:D + 1, 140 + D * l:140 + D * (l + 1)] = aug(
            out_proj_w[l].T, out_proj_b[l])
        packw[0:D + 1, 150 + F * l:150 + F * (l + 1)] = aug(
            ff1_w[l].T, ff1_b[l])
        packw[0:F + 1, 190 + D * l:190 + D * (l + 1)] = aug(
            ff2_w[l].T, ff2_b[l])
        for i, (g, bb) in enumerate(((ln1_g[l], ln1_b[l]),
                                     (ln2_g[l], ln2_b[l]))):
            k = 2 * l + i
            packw[0, 200 + D * k:200 + D * (k + 1)] = g
            packw[32, 220 + D * k:220 + D * (k + 1)] = g
            packw[0:D, 278 + k] = bb
    packw[0:D, 240] = 1.0 / D          # c2 col 0: s1 = mean(x)
    packw[32:32 + D, 272] = 1.0 / D    # c2 col 32: s2 = mean(x^2)
    packw[0:D, 273:278] = np.eye(D, dtype=np.float32)
    packw[0, 283:288] = 1.0

    # multiplicative causal mask for the diagonal 128x128 block
    idx = np.arange(128)
    maskf = (idx[None, :] >= idx[:, None]).astype(np.float32)

    fcw = np.zeros((D + 1, VP), np.float32)
    fcw[0:D, 0:V] = fc_w.T
    fcw[D, 0:V] = fc_b
    fcw = fcw.astype(ml_dtypes.bfloat16)

    ctxa0 = np.ones((D + 1, S2), np.float32)
    f1a0 = np.ones((F + 1, S2), np.float32)
    vsb0 = np.zeros((128, QC, 37), np.float32)
    vsb0[:, :, 32:37] = 1.0
    hf0 = np.ones((D + 1, S2), ml_dtypes.bfloat16)
    rr0 = np.zeros((33, S), np.float32)

    shared = dict(packw=packw, mask=np.ascontiguousarray(maskf), fcw=fcw,
                  ctxa0=ctxa0, f1a0=f1a0, vsb0=vsb0, hf0=hf0, rr0=rr0)
    in_maps = []
    for c in range(NCORES):
        hh = np.ones((D + 1, S2), np.float32)
        for b in range(BPC):
            hh[0:D, b * S:(b + 1) * S] = h0t[c * BPC + b]
        in_maps.append(dict(h0=hh, **shared))
    return in_maps


def run(in_maps, trace=False, iters=1, **kw):
    nc = _get_program(iters)
    return run_bass_kernel_spmd(nc, in_maps, list(range(NCORES)),
                                trace=trace, **kw)


def unshard(res):
    """Per-core [128, BPC, QC, VP] fp16 -> [B, S, V] fp32."""
    outs = []
    for c in range(NCORES):
        a = np.asarray(res.results[c]["out"]).astype(np.float32)
        a = a.reshape(128, BPC, QC, VP)
        a = np.transpose(a, (1, 2, 0, 3)).reshape(BPC, S, VP)[:, :, 0:V]
        outs.append(a)
    return np.ascontiguousarray(np.concatenate(outs, axis=0))


def kernel(**inputs) -> np.ndarray:
    in_maps = host_inputs(**inputs)
    res = run(in_maps)
    return unshard(res)


if __name__ == "__main__":
    import reference
    ins = {k: np.asarray(v) for k, v in reference.setup_inputs().items()}
    got = kernel(**ins)
    exp = np.asarray(reference.reference(**reference.setup_inputs()))
    err = np.abs(got - exp)
    rel = err.max() / (np.abs(exp).max() + 1e-30)
    print("max abs err:", err.max(), "rel:", rel)


# revision 47
# speedup vs baseline: 1.0023x; 1.0023x over previous
"""MicroTransformer (B=16,S=512,V=8000,D=5,F=20,L=2) on 8 trn2 NeuronCores.

Sharding: pure data parallel over batch (2 batch elements per core).
All parameters replicated. Whole transformer body + logits matmul run on
device; host only does input prep (embedding row gather, positional
encoding constant, weight layout transforms) and the final reshape.

Per-core device program (Bass/Tile, fully unrolled), v2:
  state h [6, 1024] f32r: rows 0-4 = h^T for batch0|batch1, row 5 = ones.
  qkv+vT in ONE matmul: stationary [6,69] (q scaled at cols 0-4, k at
  32-36, vT at 64-68 -> all psum reads 32-aligned).  V is transposed to
  [128k, 5] per chunk via 4 PE-transposes (identity stationary) and cast
  bf16 into the v-aug tile (ones col 32 accumulates Z).
  Attention transposed ([k,q]), softmax without row-max; exp -> bf16,
  causal mask = in-place [128,128] bf16 multiply on the diagonal block
  only (off-diagonal needs no mask).
  LayerNorm: stats via 2 matmuls into ONE psum bank (s1@row0, s2@row32,
  g*rstd@64, g*mu*rstd@96); rstd = Exp(-0.5*Ln(var+eps)) so every ACT
  function (Exp/Ln/Square/Copy/Relu-free) lives in ONE table set (no
  mid-kernel ACT table swaps).  Broadcast matmuls take a zero-initialized
  [33,512] moving tile with rstd@row0, mu*rstd@row32.
  Logits (the memory-bound phase): final h -> hfin [6,1024] bf16; fcw
  [6,8192] bf16 (vocab padded).  Flat 512-col matmuls, stationary = one
  [6,128] token chunk reused for 16 consecutive matmuls (no row-tiling,
  no fcw replicas).  Psum ring = 2x [128,1024] tiles; casts to fp16
  stage tiles ping-pong DVE/ACT; 0.5MB stores alternate the sync and
  gpsimd HWDGE rings so DMA streams continuously.  Batch 0's logits are
  emitted right after its final LN2 with batch 1's ffn tail interleaved
  mid-stream, so stores start as early as possible and never gap.
"""

import math

import numpy as np
import ml_dtypes

import concourse.bacc as bacc
import concourse.bass as bass
import concourse.mybir as mybir
import concourse.tile as tile
from concourse.bass_utils import run_bass_kernel_spmd

F32 = mybir.dt.float32
F32R = mybir.dt.float32r
BF16 = mybir.dt.bfloat16
F16 = mybir.dt.float16
ALU = mybir.AluOpType
ACTF = mybir.ActivationFunctionType


def _r(ap):
    return ap.bitcast(F32R)


def _f(ap):
    return ap.bitcast(F32)


B, S, V, D, F, L = 16, 512, 8000, 5, 20, 2
VP = 8192                      # vocab padded to 16x512
EPS = 1e-5
NCORES = 8
BPC = B // NCORES              # 2
SQRT_D = math.sqrt(float(D))
SCALE = 1.0 / SQRT_D
QC = S // 128                  # 4
S2 = BPC * S                   # 1024
NR = VP // 1024                # 8 logit rounds per (b, i)

_CACHED = {}

_ACT_SET = "natural_log_exp_and_others"


def _pin_act_tables():
    """Expose only the one table set covering Exp/Ln/Square/Copy.

    The act-table placement pass picks the first set containing each
    activation's function; with the full list it ping-pongs between the
    exp set and the ln set on every LayerNorm (measured 17 mid-kernel
    ~1.3us table loads).  Restricting the choice to the single covering
    set yields exactly one load at kernel start.
    """
    orig = bacc.get_activation_tables

    def only_covering(arch):
        t = orig(arch)
        # act_func_set_id is positional into act_info.json, so keep every
        # entry in place and just make the non-covering sets unchoosable.
        return {name: (funcs if name == _ACT_SET else set())
                for name, funcs in t.items()}

    bacc.get_activation_tables = only_covering
    return orig


def _build_program(iters=1):
    _orig_tables = _pin_act_tables()
    try:
        return _build_program_inner(iters)
    finally:
        bacc.get_activation_tables = _orig_tables


def _build_program_inner(iters=1):
    nc = bacc.Bacc("TRN2", target_bir_lowering=False, debug=False,
                   num_devices=NCORES)

    d_h0 = nc.dram_tensor("h0", [D + 1, S2], F32R, kind="ExternalInput")
    d_pack = nc.dram_tensor("packw", [37, 288], F32R, kind="ExternalInput")
    d_mask = nc.dram_tensor("mask", [128, 128], F32, kind="ExternalInput")
    d_ctxa0 = nc.dram_tensor("ctxa0", [D + 1, S2], F32R,
                             kind="ExternalInput")
    d_f1a0 = nc.dram_tensor("f1a0", [F + 1, S2], F32R, kind="ExternalInput")
    d_vsb0 = nc.dram_tensor("vsb0", [128, QC, 37], F32R,
                            kind="ExternalInput")
    d_hf0 = nc.dram_tensor("hf0", [D + 1, S2], BF16, kind="ExternalInput")
    d_rr0 = nc.dram_tensor("rr0", [33, S], F32R, kind="ExternalInput")
    d_fcw = nc.dram_tensor("fcw", [D + 1, VP], BF16, kind="ExternalInput")
    d_out = nc.dram_tensor("out", [128, BPC, QC, VP], F16,
                           kind="ExternalOutput")

    from contextlib import ExitStack
    with tile.TileContext(nc) as tc, ExitStack() as es, \
            nc.allow_low_precision(reason="bf16/f16 rounding intended"):
        cst = es.enter_context(tc.tile_pool(name="cst", bufs=1))
        wrk = es.enter_context(tc.tile_pool(name="wrk", bufs=3))
        att = es.enter_context(tc.tile_pool(name="att", bufs=3))
        stg = es.enter_context(tc.tile_pool(name="stg", bufs=6))
        # one 4-bank pool set per batch: the two body chains never share
        # psum, so they truly run in parallel; each batch's logits phase
        # then recycles pools that are idle by that point.
        ps_cz = [es.enter_context(tc.tile_pool(name=f"ps_cz{b}", bufs=1,
                                               space="PSUM"))
                 for b in range(BPC)]
        ps_ms = [es.enter_context(tc.tile_pool(name=f"ps_ms{b}", bufs=3,
                                               space="PSUM"))
                 for b in range(BPC)]

        # ---- constants ----
        h = cst.tile([D + 1, S2], F32R, name="h", tag="h")
        nc.sync.dma_start(h[:], d_h0[:])
        pack = cst.tile([37, 288], F32R, name="pack", tag="pack")
        nc.sync.dma_start(pack[:], d_pack[:])

        wqkv = [pack[0:D + 1, 70 * l:70 * l + 69] for l in range(L)]
        wo = [pack[0:D + 1, 140 + D * l:140 + D * (l + 1)] for l in range(L)]
        w1 = [pack[0:D + 1, 150 + F * l:150 + F * (l + 1)] for l in range(L)]
        w2 = [pack[0:F + 1, 190 + D * l:190 + D * (l + 1)] for l in range(L)]
        grow0 = [pack[0:33, 200 + D * k:200 + D * (k + 1)] for k in range(4)]
        grow32 = [pack[0:33, 220 + D * k:220 + D * (k + 1)] for k in range(4)]
        c02 = pack[0:D, 240:241]       # stats stationary (1/D column)
        eye5 = _f(pack[0:D, 273:278])
        bcol = [_f(pack[0:D, 278 + k:279 + k]) for k in range(4)]
        ones5 = pack[0:1, 283:288]

        maskb = cst.tile([128, 128], F32, name="maskb", tag="maskb")
        nc.gpsimd.dma_start(maskb[:], d_mask[:])
        c30 = cst.tile([128, 1], F32, name="c30", tag="c30")
        nc.vector.memset(c30[:], -30.0)
        ctxa = cst.tile([D + 1, S2], F32R, name="ctxa", tag="ctxa")
        nc.gpsimd.dma_start(ctxa[:], d_ctxa0[:])
        f1a = cst.tile([F + 1, S2], F32R, name="f1a", tag="f1a")
        nc.gpsimd.dma_start(f1a[:], d_f1a0[:])
        vsb = []
        for b in range(BPC):
            t = cst.tile([128, QC, 37], F32R, name=f"vsb{b}", tag=f"vsb{b}")
            nc.gpsimd.dma_start(t[:], d_vsb0[:])
            vsb.append(t)
        rr = []
        for b in range(BPC):
            t = cst.tile([33, S], F32R, name=f"rr{b}", tag=f"rr{b}")
            nc.gpsimd.dma_start(t[:], d_rr0[:])
            rr.append(t)
        hfin = cst.tile([D + 1, S2], BF16, name="hfin", tag="hfin")
        nc.gpsimd.dma_start(hfin[:], d_hf0[:])
        fcw = cst.tile([D + 1, VP], BF16, name="fcw", tag="fcw")
        nc.gpsimd.dma_start(fcw[:], d_fcw[:])

        def layernorm(l, i, b, it, resid_ap, add_ps, out_ap):
            """out = LN(resid + add_ps) * g + b.

            Two independent 256-token half-chains; halving the free size
            halves every op's duration, and the halves pipeline across
            engines, roughly halving the ~13-step serial latency.
            """
            u = f"{l}{i}{b}_{it}"
            k = 2 * l + i
            HS = S // 2
            x = wrk.tile([D, S], F32R, name=f"lx{u}", tag="lx")
            xsq = wrk.tile([D, S], F32R, name=f"lq{u}", tag="lq")
            s1 = ps_ms[b].tile([1, S], F32, name=f"s1{u}", tag="ps_ms")
            s2 = ps_ms[b].tile([1, S], F32, name=f"s2{u}", tag="ps_ms")
            t1 = wrk.tile([1, S], F32, name=f"t1{u}", tag="lt1")
            var = wrk.tile([1, S], F32, name=f"lv{u}", tag="lvar")
            lnv = wrk.tile([1, S], F32, name=f"ll{u}", tag="llnv")
            rbp = ps_ms[b].tile([D, S], F32, name=f"lr{u}", tag="ps_ms")
            rmp = ps_ms[b].tile([D, S], F32, name=f"lm{u}", tag="ps_ms")
            t2 = wrk.tile([D, S], F32, name=f"t2{u}", tag="lt2")
            hs = [slice(0, HS), slice(HS, S)]
            for hh in hs:
                nc.vector.tensor_add(x[:, hh], resid_ap[:, hh],
                                     add_ps[:, hh])
            for hh in hs:
                nc.gpsimd.tensor_mul(xsq[:, hh], x[:, hh], x[:, hh])
                nc.tensor.matmul(s1[:, hh], c02, x[:, hh])
            for hh in hs:
                nc.tensor.matmul(s2[:, hh], c02, xsq[:, hh])
                nc.scalar.square(t1[:, hh], s1[:, hh])
            for hh in hs:
                nc.vector.scalar_tensor_tensor(var[:, hh], s2[:, hh], EPS,
                                               t1[:, hh],
                                               op0=ALU.add, op1=ALU.subtract)
            for hh in hs:
                nc.scalar.activation(lnv[:, hh], var[:, hh], ACTF.Ln)
            for hh in hs:
                nc.scalar.activation(rr[b][0:1, hh], lnv[:, hh], ACTF.Exp,
                                     scale=-0.5)
                nc.vector.tensor_mul(rr[b][32:33, hh], _f(rr[b][0:1, hh]),
                                     s1[:, hh])
            for hh in hs:
                nc.tensor.matmul(rbp[:, hh], grow0[k], rr[b][:, hh])
                nc.tensor.matmul(rmp[:, hh], grow32[k], rr[b][:, hh])
            for hh in hs:
                nc.vector.tensor_mul(t2[:, hh], _f(x[:, hh]), rbp[:, hh])
            for hh in hs:
                nc.vector.scalar_tensor_tensor(out_ap[:, hh], t2[:, hh],
                                               bcol[k], rmp[:, hh],
                                               op0=ALU.add, op1=ALU.subtract)

        def attn(l, b, it):
            u = f"{l}{b}_{it}"
            hb = h[:, b * S:(b + 1) * S]
            qkp = ps_ms[b].tile([69, S], F32, name=f"qk{u}", tag="ps_ms")
            nc.tensor.matmul(qkp[:], wqkv[l], hb)
            qsb = wrk.tile([D, S], F32R, name=f"qs{u}", tag="qsb")
            ksb = wrk.tile([D, S], F32R, name=f"ks{u}", tag="ksb")
            vTs = wrk.tile([D, S], F32, name=f"vt{u}", tag="vts")
            nc.scalar.copy(qsb[:], qkp[0:D, :])
            nc.vector.tensor_copy(ksb[:], qkp[32:32 + D, :])
            nc.scalar.copy(vTs[:], qkp[64:64 + D, :])
            # v -> [128k, 5] per chunk via PE transpose; cast bf16 into v-aug
            # (8-wide chunk stride keeps each transpose out 32B-aligned)
            vtp = ps_ms[b].tile([128, QC, 8], F32, name=f"vp{u}", tag="ps_ms")
            for kc in range(QC):
                nc.tensor.transpose(vtp[:, kc, 0:D],
                                    vTs[:, kc * 128:(kc + 1) * 128], eye5)
            nc.vector.tensor_copy(vsb[b][:, :, 0:D], vtp[:, :, 0:D])

            cz = ps_cz[b].tile([37, S], F32, name=f"cz{u}", tag="ps_cz")
            for kc in range(QC):
                off = kc * 128
                n = S - off
                scp = ps_ms[b].tile([128, S], F32, name=f"sc{u}{kc}",
                                    tag="ps_ms")
                nc.tensor.matmul(scp[:, 0:n], ksb[:, off:off + 128],
                                 qsb[:, off:S])
                expT = att.tile([128, S], F32R, name=f"ex{u}{kc}", tag="expT")
                # softmax is shift-invariant: exp(s-30) keeps the table's
                # input/output range comfortable (scores reach ~|50|)
                nc.scalar.activation(expT[:, 0:n], scp[:, 0:n], ACTF.Exp,
                                     bias=c30[:])
                nc.gpsimd.tensor_mul(expT[:, 0:128], expT[:, 0:128],
                                     maskb[:])
                nc.tensor.matmul(cz[:, off:S], vsb[b][:, kc, :],
                                 expT[:, 0:n],
                                 start=(kc == 0), stop=(kc == QC - 1))
            zf = wrk.tile([D, S], F32, name=f"zf{u}", tag="zf")
            nc.scalar.copy(zf[:], cz[32:32 + D, :])
            ctxs = wrk.tile([D, S], F32, name=f"cs{u}", tag="ctxs")
            nc.scalar.copy(ctxs[:], cz[0:D, :])
            zr = wrk.tile([D, S], F32, name=f"zr{u}", tag="zr")
            nc.vector.reciprocal_approx_fast(zr[:], zf[:])
            nc.gpsimd.tensor_mul(ctxa[0:D, b * S:(b + 1) * S], ctxs[:],
                                 zr[:])

        def proj_ln1(l, b, it):
            bc0 = b * S
            pp = ps_ms[b].tile([D, S], F32, name=f"pp{l}{b}_{it}", tag="ps_ms")
            nc.tensor.matmul(pp[:], wo[l], ctxa[:, bc0:bc0 + S])
            layernorm(l, 0, b, it, h[0:D, bc0:bc0 + S], pp[:],
                      h[0:D, bc0:bc0 + S])

        def ffn_ln2(l, b, it):
            bc0 = b * S
            f1p = ps_ms[b].tile([F, S], F32, name=f"f1{l}{b}_{it}", tag="ps_ms")
            nc.tensor.matmul(f1p[:], w1[l], h[:, bc0:bc0 + S])
            nc.vector.tensor_scalar_max(f1a[0:F, bc0:bc0 + S // 2],
                                        f1p[:, 0:S // 2], 0.0)
            nc.vector.tensor_scalar_max(f1a[0:F, bc0 + S // 2:bc0 + S],
                                        f1p[:, S // 2:S], 0.0)
            f2p = ps_ms[b].tile([D, S], F32, name=f"f2{l}{b}_{it}", tag="ps_ms")
            nc.tensor.matmul(f2p[:], w2[l], f1a[:, bc0:bc0 + S])
            if l == L - 1:
                layernorm(l, 1, b, it, h[0:D, bc0:bc0 + S], f2p[:],
                          hfin[0:D, bc0:bc0 + S])
            else:
                layernorm(l, 1, b, it, h[0:D, bc0:bc0 + S], f2p[:],
                          h[0:D, bc0:bc0 + S])

        def logits_b(b, it, tails=None):
            """32 rounds of [2 matmuls -> cast -> staged 0.5MB store].

            tails: {round_index: callback} -- emits the other batch's body
            segments between rounds so its work drains during this batch's
            store stream (engine queues are in-order; segments must be
            small enough not to starve the cast/store pipeline).
            """
            # this batch's own body pools are free once hfin is ready;
            # batch 1's logits can additionally recycle batch 0's pools
            # (its logits phase has completed by then) for a deeper ring.
            ring = [(ps_cz[b], "ps_cz"), (ps_ms[b], "ps_ms"),
                    (ps_ms[b], "ps_ms"), (ps_ms[b], "ps_ms")]
            if b == 1:
                ring = ring + [(ps_cz[0], "ps_cz"), (ps_ms[0], "ps_ms"),
                               (ps_ms[0], "ps_ms"), (ps_ms[0], "ps_ms")]
            nr = 0
            st = None
            for i in range(QC):
                stat = hfin[0:D + 1, b * S + 128 * i:b * S + 128 * (i + 1)]
                for r in range(VP // 512):
                    pool, tg = ring[nr % len(ring)]
                    lp = pool.tile([128, 512], F32,
                                   name=f"lp{b}{i}{r}_{it}", tag=tg)
                    v0 = r * 512
                    nc.tensor.matmul(lp[:], stat, fcw[:, v0:v0 + 512])
                    if r % 4 == 0:
                        st = stg.tile([128, 2048], F16,
                                      name=f"st{b}{i}{r}_{it}", tag="stage")
                    sl = st[:, (r % 4) * 512:(r % 4) * 512 + 512]
                    if nr % 2 == 0:
                        nc.vector.tensor_copy(sl, lp[:])
                    else:
                        nc.scalar.copy(sl, lp[:])
                    if r % 4 == 3:
                        eng = nc.sync if (nr // 4) % 2 == 0 else nc.gpsimd
                        eng.dma_start(d_out[:, b, i, v0 - 1536:v0 + 512],
                                      st[:])
                    nr += 1
                    if tails and nr in tails:
                        tails[nr]()

        for it in range(iters):
            if it > 0:
                nc.sync.dma_start(h[:], d_h0[:])
            # stepwise batch interleave: the two chains fill each other's
            # engine gaps, and both logits phases then run contention-free
            # (pure matmul+cast+store streams).
            for l in range(L):
                for b in range(BPC):
                    attn(l, b, it)
                for b in range(BPC):
                    proj_ln1(l, b, it)
                if l < L - 1:
                    for b in range(BPC):
                        ffn_ln2(l, b, it)
            # final-layer tail: emit batch 0's ffn+LN2, then its logits with
            # batch 1's ffn+LN2 injected mid-stream so batch 1's hfin is
            # ready just before its own logits rounds begin.
            for b in range(BPC):
                ffn_ln2(L - 1, b, it)
            logits_b(0, it)
            logits_b(1, it)

    nc.compile()
    return nc


def _get_program(iters=1):
    if iters not in _CACHED:
        _CACHED[iters] = _build_program(iters)
    return _CACHED[iters]


def _pos_encoding_np():
    pos = np.arange(B, dtype=np.float32)[:, None]
    div = np.exp(np.arange(0, D, 2, dtype=np.float32)
                 * (-math.log(10000.0) / D))
    pe = np.zeros((B, D), dtype=np.float32)
    pe[:, 0::2] = np.sin(pos * div)
    pe[:, 1::2] = np.cos(pos * div[:-1])
    return pe


def host_inputs(x, emb, in_proj_w, in_proj_b, out_proj_w, out_proj_b,
                ln1_g, ln1_b, ln2_g, ln2_b, ff1_w, ff1_b, ff2_w, ff2_b,
                fc_w, fc_b):
    x = np.asarray(x).astype(np.int64)
    emb = np.asarray(emb, dtype=np.float32)
    f32 = lambda a: np.ascontiguousarray(np.asarray(a, dtype=np.float32))
    in_proj_w, in_proj_b = f32(in_proj_w), f32(in_proj_b)
    out_proj_w, out_proj_b = f32(out_proj_w), f32(out_proj_b)
    ff1_w, ff1_b, ff2_w, ff2_b = f32(ff1_w), f32(ff1_b), f32(ff2_w), f32(ff2_b)
    ln1_g, ln1_b, ln2_g, ln2_b = f32(ln1_g), f32(ln1_b), f32(ln2_g), f32(ln2_b)
    fc_w, fc_b = f32(fc_w), f32(fc_b)

    h0 = emb[x] * np.float32(SQRT_D)
    h0 = h0 + _pos_encoding_np()[:, None, :]
    h0t = np.transpose(h0, (0, 2, 1))          # [B, D, S]

    def aug(wT, bias):
        return np.concatenate([wT, bias[None, :]], axis=0).astype(np.float32)

    packw = np.zeros((37, 288), np.float32)
    for l in range(L):
        packw[0:D + 1, 70 * l:70 * l + D] = aug(
            in_proj_w[l][0:D].T * SCALE, in_proj_b[l][0:D] * SCALE)
        packw[0:D + 1, 70 * l + 32:70 * l + 32 + D] = aug(
            in_proj_w[l][D:2 * D].T, in_proj_b[l][D:2 * D])
        packw[0:D + 1, 70 * l + 64:70 * l + 64 + D] = aug(
            in_proj_w[l][2 * D:3 * D].T, in_proj_b[l][2 * D:3 * D])
        packw[0:D + 1, 140 + D * l:140 + D * (l + 1)] = aug(
            out_proj_w[l].T, out_proj_b[l])
        packw[0:D + 1, 150 + F * l:150 + F * (l + 1)] = aug(
            ff1_w[l].T, ff1_b[l])
        packw[0:F + 1, 190 + D * l:190 + D * (l + 1)] = aug(
            ff2_w[l].T, ff2_b[l])
        for i, (g, bb) in enumerate(((ln1_g[l], ln1_b[l]),
                                     (ln2_g[l], ln2_b[l]))):
            k = 2 * l + i
            packw[0, 200 + D * k:200 + D * (k + 1)] = g
            packw[32, 220 + D * k:220 + D * (k + 1)] = g
            packw[0:D, 278 + k] = bb
    packw[0:D, 240] = 1.0 / D          # c2 col 0: s1 = mean(x)
    packw[32:32 + D, 272] = 1.0 / D    # c2 col 32: s2 = mean(x^2)
    packw[0:D, 273:278] = np.eye(D, dtype=np.float32)
    packw[0, 283:288] = 1.0

    # multiplicative causal mask for the diagonal 128x128 block
    idx = np.arange(128)
    maskf = (idx[None, :] >= idx[:, None]).astype(np.float32)

    fcw = np.zeros((D + 1, VP), np.float32)
    fcw[0:D, 0:V] = fc_w.T
    fcw[D, 0:V] = fc_b
    fcw = fcw.astype(ml_dtypes.bfloat16)

    ctxa0 = np.ones((D + 1, S2), np.float32)
    f1a0 = np.ones((F + 1, S2), np.float32)
    vsb0 = np.zeros((128, QC, 37), np.float32)
    vsb0[:, :, 32:37] = 1.0
    hf0 = np.ones((D + 1, S2), ml_dtypes.bfloat16)
    rr0 = np.zeros((33, S), np.float32)

    shared = dict(packw=packw, mask=np.ascontiguousarray(maskf), fcw=fcw,
                  ctxa0=ctxa0, f1a0=f1a0, vsb0=vsb0, hf0=hf0, rr0=rr0)
    in_maps = []
    for c in range(NCORES):
        hh = np.ones((D + 1, S2), np.float32)
        for b in range(BPC):
            hh[0:D, b * S:(b + 1) * S] = h0t[c * BPC + b]
        in_maps.append(dict(h0=hh, **shared))
    return in_maps


def run(in_maps, trace=False, iters=1, **kw):
    nc = _get_program(iters)
    return run_bass_kernel_spmd(nc, in_maps, list(range(NCORES)),
                                trace=trace, **kw)


def unshard(res):
    """Per-core [128, BPC, QC, VP] fp16 -> [B, S, V] fp32."""
    outs = []
    for c in range(NCORES):
        a = np.asarray(res.results[c]["out"]).astype(np.float32)
        a = a.reshape(128, BPC, QC, VP)
        a = np.transpose(a, (1, 2, 0, 3)).reshape(BPC, S, VP)[:, :, 0:V]
        outs.append(a)
    return np.ascontiguousarray(np.concatenate(outs, axis=0))


def kernel(**inputs) -> np.ndarray:
    in_maps = host_inputs(**inputs)
    res = run(in_maps)
    return unshard(res)


if __name__ == "__main__":
    import reference
    ins = {k: np.asarray(v) for k, v in reference.setup_inputs().items()}
    got = kernel(**ins)
    exp = np.asarray(reference.reference(**reference.setup_inputs()))
    err = np.abs(got - exp)
    rel = err.max() / (np.abs(exp).max() + 1e-30)
    print("max abs err:", err.max(), "rel:", rel)


# revision 48
# speedup vs baseline: 1.0029x; 1.0006x over previous
"""MicroTransformer (B=16,S=512,V=8000,D=5,F=20,L=2) on 8 trn2 NeuronCores.

Sharding: pure data parallel over batch (2 batch elements per core).
All parameters replicated. Whole transformer body + logits matmul run on
device; host only does input prep (embedding row gather, positional
encoding constant, weight layout transforms) and the final reshape.

Per-core device program (Bass/Tile, fully unrolled), v2:
  state h [6, 1024] f32r: rows 0-4 = h^T for batch0|batch1, row 5 = ones.
  qkv+vT in ONE matmul: stationary [6,69] (q scaled at cols 0-4, k at
  32-36, vT at 64-68 -> all psum reads 32-aligned).  V is transposed to
  [128k, 5] per chunk via 4 PE-transposes (identity stationary) and cast
  bf16 into the v-aug tile (ones col 32 accumulates Z).
  Attention transposed ([k,q]), softmax without row-max; exp -> bf16,
  causal mask = in-place [128,128] bf16 multiply on the diagonal block
  only (off-diagonal needs no mask).
  LayerNorm: stats via 2 matmuls into ONE psum bank (s1@row0, s2@row32,
  g*rstd@64, g*mu*rstd@96); rstd = Exp(-0.5*Ln(var+eps)) so every ACT
  function (Exp/Ln/Square/Copy/Relu-free) lives in ONE table set (no
  mid-kernel ACT table swaps).  Broadcast matmuls take a zero-initialized
  [33,512] moving tile with rstd@row0, mu*rstd@row32.
  Logits (the memory-bound phase): final h -> hfin [6,1024] bf16; fcw
  [6,8192] bf16 (vocab padded).  Flat 512-col matmuls, stationary = one
  [6,128] token chunk reused for 16 consecutive matmuls (no row-tiling,
  no fcw replicas).  Psum ring = 2x [128,1024] tiles; casts to fp16
  stage tiles ping-pong DVE/ACT; 0.5MB stores alternate the sync and
  gpsimd HWDGE rings so DMA streams continuously.  Batch 0's logits are
  emitted right after its final LN2 with batch 1's ffn tail interleaved
  mid-stream, so stores start as early as possible and never gap.
"""

import math

import numpy as np
import ml_dtypes

import concourse.bacc as bacc
import concourse.bass as bass
import concourse.mybir as mybir
import concourse.tile as tile
from concourse.bass_utils import run_bass_kernel_spmd

F32 = mybir.dt.float32
F32R = mybir.dt.float32r
BF16 = mybir.dt.bfloat16
F16 = mybir.dt.float16
ALU = mybir.AluOpType
ACTF = mybir.ActivationFunctionType


def _r(ap):
    return ap.bitcast(F32R)


def _f(ap):
    return ap.bitcast(F32)


B, S, V, D, F, L = 16, 512, 8000, 5, 20, 2
VP = 8192                      # vocab padded to 16x512
EPS = 1e-5
NCORES = 8
BPC = B // NCORES              # 2
SQRT_D = math.sqrt(float(D))
SCALE = 1.0 / SQRT_D
QC = S // 128                  # 4
S2 = BPC * S                   # 1024
NR = VP // 1024                # 8 logit rounds per (b, i)

_CACHED = {}

_ACT_SET = "natural_log_exp_and_others"


def _pin_act_tables():
    """Expose only the one table set covering Exp/Ln/Square/Copy.

    The act-table placement pass picks the first set containing each
    activation's function; with the full list it ping-pongs between the
    exp set and the ln set on every LayerNorm (measured 17 mid-kernel
    ~1.3us table loads).  Restricting the choice to the single covering
    set yields exactly one load at kernel start.
    """
    orig = bacc.get_activation_tables

    def only_covering(arch):
        t = orig(arch)
        # act_func_set_id is positional into act_info.json, so keep every
        # entry in place and just make the non-covering sets unchoosable.
        return {name: (funcs if name == _ACT_SET else set())
                for name, funcs in t.items()}

    bacc.get_activation_tables = only_covering
    return orig


def _build_program(iters=1):
    _orig_tables = _pin_act_tables()
    try:
        return _build_program_inner(iters)
    finally:
        bacc.get_activation_tables = _orig_tables


def _build_program_inner(iters=1):
    nc = bacc.Bacc("TRN2", target_bir_lowering=False, debug=False,
                   num_devices=NCORES)

    d_h0 = nc.dram_tensor("h0", [D + 1, S2], F32R, kind="ExternalInput")
    d_pack = nc.dram_tensor("packw", [37, 288], F32R, kind="ExternalInput")
    d_mask = nc.dram_tensor("mask", [128, 128], F32, kind="ExternalInput")
    d_ctxa0 = nc.dram_tensor("ctxa0", [D + 1, S2], F32R,
                             kind="ExternalInput")
    d_f1a0 = nc.dram_tensor("f1a0", [F + 1, S2], F32R, kind="ExternalInput")
    d_vsb0 = nc.dram_tensor("vsb0", [128, QC, 37], F32R,
                            kind="ExternalInput")
    d_hf0 = nc.dram_tensor("hf0", [D + 1, S2], BF16, kind="ExternalInput")
    d_rr0 = nc.dram_tensor("rr0", [33, S], F32R, kind="ExternalInput")
    d_fcw = nc.dram_tensor("fcw", [D + 1, VP], BF16, kind="ExternalInput")
    d_out = nc.dram_tensor("out", [128, BPC, QC, VP], F16,
                           kind="ExternalOutput")

    from contextlib import ExitStack
    with tile.TileContext(nc) as tc, ExitStack() as es, \
            nc.allow_low_precision(reason="bf16/f16 rounding intended"):
        cst = es.enter_context(tc.tile_pool(name="cst", bufs=1))
        wrk = es.enter_context(tc.tile_pool(name="wrk", bufs=3))
        att = es.enter_context(tc.tile_pool(name="att", bufs=3))
        stg = es.enter_context(tc.tile_pool(name="stg", bufs=6))
        # one 4-bank pool set per batch: the two body chains never share
        # psum, so they truly run in parallel; each batch's logits phase
        # then recycles pools that are idle by that point.
        ps_cz = [es.enter_context(tc.tile_pool(name=f"ps_cz{b}", bufs=1,
                                               space="PSUM"))
                 for b in range(BPC)]
        ps_ms = [es.enter_context(tc.tile_pool(name=f"ps_ms{b}", bufs=3,
                                               space="PSUM"))
                 for b in range(BPC)]

        # ---- constants ----
        h = cst.tile([D + 1, S2], F32R, name="h", tag="h")
        nc.sync.dma_start(h[:], d_h0[:])
        pack = cst.tile([37, 288], F32R, name="pack", tag="pack")
        nc.sync.dma_start(pack[:], d_pack[:])

        wqkv = [pack[0:D + 1, 70 * l:70 * l + 69] for l in range(L)]
        wo = [pack[0:D + 1, 140 + D * l:140 + D * (l + 1)] for l in range(L)]
        w1 = [pack[0:D + 1, 150 + F * l:150 + F * (l + 1)] for l in range(L)]
        w2 = [pack[0:F + 1, 190 + D * l:190 + D * (l + 1)] for l in range(L)]
        grow0 = [pack[0:33, 200 + D * k:200 + D * (k + 1)] for k in range(4)]
        grow32 = [pack[0:33, 220 + D * k:220 + D * (k + 1)] for k in range(4)]
        c02 = pack[0:D, 240:241]       # stats stationary (1/D column)
        eye5 = _f(pack[0:D, 273:278])
        bcol = [_f(pack[0:D, 278 + k:279 + k]) for k in range(4)]
        ones5 = pack[0:1, 283:288]

        maskb = cst.tile([128, 128], F32, name="maskb", tag="maskb")
        nc.gpsimd.dma_start(maskb[:], d_mask[:])
        c30 = cst.tile([128, 1], F32, name="c30", tag="c30")
        nc.vector.memset(c30[:], -30.0)
        ctxa = cst.tile([D + 1, S2], F32R, name="ctxa", tag="ctxa")
        nc.gpsimd.dma_start(ctxa[:], d_ctxa0[:])
        f1a = cst.tile([F + 1, S2], F32R, name="f1a", tag="f1a")
        nc.gpsimd.dma_start(f1a[:], d_f1a0[:])
        vsb = []
        for b in range(BPC):
            t = cst.tile([128, QC, 37], F32R, name=f"vsb{b}", tag=f"vsb{b}")
            nc.gpsimd.dma_start(t[:], d_vsb0[:])
            vsb.append(t)
        rr = []
        for b in range(BPC):
            t = cst.tile([33, S], F32R, name=f"rr{b}", tag=f"rr{b}")
            nc.gpsimd.dma_start(t[:], d_rr0[:])
            rr.append(t)
        hfin = cst.tile([D + 1, S2], BF16, name="hfin", tag="hfin")
        nc.gpsimd.dma_start(hfin[:], d_hf0[:])
        fcw = cst.tile([D + 1, VP], BF16, name="fcw", tag="fcw")
        nc.gpsimd.dma_start(fcw[:], d_fcw[:])

        def layernorm(l, i, b, it, resid_ap, add_ps, out_ap):
            """out = LN(resid + add_ps) * g + b.

            Two independent 256-token half-chains; halving the free size
            halves every op's duration, and the halves pipeline across
            engines, roughly halving the ~13-step serial latency.
            """
            u = f"{l}{i}{b}_{it}"
            k = 2 * l + i
            HS = S // 2
            x = wrk.tile([D, S], F32R, name=f"lx{u}", tag="lx")
            xsq = wrk.tile([D, S], F32R, name=f"lq{u}", tag="lq")
            s1 = ps_ms[b].tile([1, S], F32, name=f"s1{u}", tag="ps_ms")
            s2 = ps_ms[b].tile([1, S], F32, name=f"s2{u}", tag="ps_ms")
            t1 = wrk.tile([1, S], F32, name=f"t1{u}", tag="lt1")
            var = wrk.tile([1, S], F32, name=f"lv{u}", tag="lvar")
            lnv = wrk.tile([1, S], F32, name=f"ll{u}", tag="llnv")
            rbp = ps_ms[b].tile([D, S], F32, name=f"lr{u}", tag="ps_ms")
            rmp = ps_ms[b].tile([D, S], F32, name=f"lm{u}", tag="ps_ms")
            t2 = wrk.tile([D, S], F32, name=f"t2{u}", tag="lt2")
            hs = [slice(0, HS), slice(HS, S)]
            for hh in hs:
                nc.vector.tensor_add(x[:, hh], resid_ap[:, hh],
                                     add_ps[:, hh])
            for hh in hs:
                nc.gpsimd.tensor_mul(xsq[:, hh], x[:, hh], x[:, hh])
                nc.tensor.matmul(s1[:, hh], c02, x[:, hh])
            for hh in hs:
                nc.tensor.matmul(s2[:, hh], c02, xsq[:, hh])
                nc.scalar.square(t1[:, hh], s1[:, hh])
            for hh in hs:
                nc.vector.scalar_tensor_tensor(var[:, hh], s2[:, hh], EPS,
                                               t1[:, hh],
                                               op0=ALU.add, op1=ALU.subtract)
            for hh in hs:
                nc.scalar.activation(lnv[:, hh], var[:, hh], ACTF.Ln)
            for hh in hs:
                nc.scalar.activation(rr[b][0:1, hh], lnv[:, hh], ACTF.Exp,
                                     scale=-0.5)
                nc.vector.tensor_mul(rr[b][32:33, hh], _f(rr[b][0:1, hh]),
                                     s1[:, hh])
            for hh in hs:
                nc.tensor.matmul(rbp[:, hh], grow0[k], rr[b][:, hh])
                nc.tensor.matmul(rmp[:, hh], grow32[k], rr[b][:, hh])
            for hh in hs:
                nc.vector.tensor_mul(t2[:, hh], _f(x[:, hh]), rbp[:, hh])
            for hh in hs:
                nc.vector.scalar_tensor_tensor(out_ap[:, hh], t2[:, hh],
                                               bcol[k], rmp[:, hh],
                                               op0=ALU.add, op1=ALU.subtract)

        def attn(l, b, it):
            u = f"{l}{b}_{it}"
            hb = h[:, b * S:(b + 1) * S]
            qkp = ps_ms[b].tile([69, S], F32, name=f"qk{u}", tag="ps_ms")
            nc.tensor.matmul(qkp[:], wqkv[l], hb)
            qsb = wrk.tile([D, S], F32R, name=f"qs{u}", tag="qsb")
            ksb = wrk.tile([D, S], F32R, name=f"ks{u}", tag="ksb")
            vTs = wrk.tile([D, S], F32, name=f"vt{u}", tag="vts")
            nc.scalar.copy(qsb[:], qkp[0:D, :])
            nc.vector.tensor_copy(ksb[:], qkp[32:32 + D, :])
            # scores first (chain-critical); the v path fills engine gaps
            sc_tiles = []
            expTs = []
            for kc in range(QC):
                off = kc * 128
                n = S - off
                scp = ps_ms[b].tile([128, S], F32, name=f"sc{u}{kc}",
                                    tag="ps_ms")
                nc.tensor.matmul(scp[:, 0:n], ksb[:, off:off + 128],
                                 qsb[:, off:S])
                sc_tiles.append(scp)
                if kc == 0:
                    nc.scalar.copy(vTs[:], qkp[64:64 + D, :])
            expQ = []
            vtp = ps_ms[b].tile([128, QC, 8], F32, name=f"vp{u}", tag="ps_ms")
            for kc in range(QC):
                off = kc * 128
                n = S - off
                scp = sc_tiles[kc]
                expT = att.tile([128, S], F32R, name=f"ex{u}{kc}", tag="expT")
                # softmax is shift-invariant: exp(s-30) keeps the table's
                # input/output range comfortable (scores reach ~|50|)
                nc.scalar.activation(expT[:, 0:n], scp[:, 0:n], ACTF.Exp,
                                     bias=c30[:])
                nc.gpsimd.tensor_mul(expT[:, 0:128], expT[:, 0:128],
                                     maskb[:])
                expQ.append(expT)
                if kc == 0:
                    for t in range(QC):
                        nc.tensor.transpose(vtp[:, t, 0:D],
                                            vTs[:, t * 128:(t + 1) * 128],
                                            eye5)
                    nc.vector.tensor_copy(vsb[b][:, :, 0:D], vtp[:, :, 0:D])
            cz = ps_cz[b].tile([37, S], F32, name=f"cz{u}", tag="ps_cz")
            for kc in range(QC):
                off = kc * 128
                n = S - off
                nc.tensor.matmul(cz[:, off:S], vsb[b][:, kc, :],
                                 expQ[kc][:, 0:n],
                                 start=(kc == 0), stop=(kc == QC - 1))
            zf = wrk.tile([D, S], F32, name=f"zf{u}", tag="zf")
            nc.scalar.copy(zf[:], cz[32:32 + D, :])
            ctxs = wrk.tile([D, S], F32, name=f"cs{u}", tag="ctxs")
            nc.vector.tensor_copy(ctxs[:], cz[0:D, :])
            zr = wrk.tile([D, S], F32, name=f"zr{u}", tag="zr")
            nc.vector.reciprocal_approx_fast(zr[:], zf[:])
            nc.gpsimd.tensor_mul(ctxa[0:D, b * S:(b + 1) * S], ctxs[:],
                                 zr[:])

        def proj_ln1(l, b, it):
            bc0 = b * S
            pp = ps_ms[b].tile([D, S], F32, name=f"pp{l}{b}_{it}", tag="ps_ms")
            nc.tensor.matmul(pp[:], wo[l], ctxa[:, bc0:bc0 + S])
            layernorm(l, 0, b, it, h[0:D, bc0:bc0 + S], pp[:],
                      h[0:D, bc0:bc0 + S])

        def ffn_ln2(l, b, it):
            bc0 = b * S
            f1p = ps_ms[b].tile([F, S], F32, name=f"f1{l}{b}_{it}", tag="ps_ms")
            nc.tensor.matmul(f1p[:], w1[l], h[:, bc0:bc0 + S])
            nc.vector.tensor_scalar_max(f1a[0:F, bc0:bc0 + S // 2],
                                        f1p[:, 0:S // 2], 0.0)
            nc.vector.tensor_scalar_max(f1a[0:F, bc0 + S // 2:bc0 + S],
                                        f1p[:, S // 2:S], 0.0)
            f2p = ps_ms[b].tile([D, S], F32, name=f"f2{l}{b}_{it}", tag="ps_ms")
            nc.tensor.matmul(f2p[:], w2[l], f1a[:, bc0:bc0 + S])
            if l == L - 1:
                layernorm(l, 1, b, it, h[0:D, bc0:bc0 + S], f2p[:],
                          hfin[0:D, bc0:bc0 + S])
            else:
                layernorm(l, 1, b, it, h[0:D, bc0:bc0 + S], f2p[:],
                          h[0:D, bc0:bc0 + S])

        def logits_b(b, it, tails=None):
            """32 rounds of [2 matmuls -> cast -> staged 0.5MB store].

            tails: {round_index: callback} -- emits the other batch's body
            segments between rounds so its work drains during this batch's
            store stream (engine queues are in-order; segments must be
            small enough not to starve the cast/store pipeline).
            """
            # this batch's own body pools are free once hfin is ready;
            # batch 1's logits can additionally recycle batch 0's pools
            # (its logits phase has completed by then) for a deeper ring.
            ring = [(ps_cz[b], "ps_cz"), (ps_ms[b], "ps_ms"),
                    (ps_ms[b], "ps_ms"), (ps_ms[b], "ps_ms"),
                    (ps_cz[1 - b], "ps_cz")]
            if b == 1:
                ring = ring + [(ps_cz[0], "ps_cz"), (ps_ms[0], "ps_ms"),
                               (ps_ms[0], "ps_ms"), (ps_ms[0], "ps_ms")]
            nr = 0
            st = None
            for i in range(QC):
                stat = hfin[0:D + 1, b * S + 128 * i:b * S + 128 * (i + 1)]
                for r in range(VP // 512):
                    pool, tg = ring[nr % len(ring)]
                    lp = pool.tile([128, 512], F32,
                                   name=f"lp{b}{i}{r}_{it}", tag=tg)
                    v0 = r * 512
                    nc.tensor.matmul(lp[:], stat, fcw[:, v0:v0 + 512])
                    if r % 4 == 0:
                        st = stg.tile([128, 2048], F16,
                                      name=f"st{b}{i}{r}_{it}", tag="stage")
                    sl = st[:, (r % 4) * 512:(r % 4) * 512 + 512]
                    if nr % 2 == 0:
                        nc.vector.tensor_copy(sl, lp[:])
                    else:
                        nc.scalar.copy(sl, lp[:])
                    if r % 4 == 3:
                        eng = nc.sync if (nr // 4) % 2 == 0 else nc.gpsimd
                        eng.dma_start(d_out[:, b, i, v0 - 1536:v0 + 512],
                                      st[:])
                    nr += 1
                    if tails and nr in tails:
                        tails[nr]()

        for it in range(iters):
            if it > 0:
                nc.sync.dma_start(h[:], d_h0[:])
            # stepwise batch interleave: the two chains fill each other's
            # engine gaps, and both logits phases then run contention-free
            # (pure matmul+cast+store streams).
            for l in range(L):
                for b in range(BPC):
                    attn(l, b, it)
                for b in range(BPC):
                    proj_ln1(l, b, it)
                if l < L - 1:
                    for b in range(BPC):
                        ffn_ln2(l, b, it)
            # final-layer tail: emit batch 0's ffn+LN2, then its logits with
            # batch 1's ffn+LN2 injected mid-stream so batch 1's hfin is
            # ready just before its own logits rounds begin.
            for b in range(BPC):
                ffn_ln2(L - 1, b, it)
            logits_b(0, it)
            logits_b(1, it)

    nc.compile()
    return nc


def _get_program(iters=1):
    if iters not in _CACHED:
        _CACHED[iters] = _build_program(iters)
    return _CACHED[iters]


def _pos_encoding_np():
    pos = np.arange(B, dtype=np.float32)[:, None]
    div = np.exp(np.arange(0, D, 2, dtype=np.float32)
                 * (-math.log(10000.0) / D))
    pe = np.zeros((B, D), dtype=np.float32)
    pe[:, 0::2] = np.sin(pos * div)
    pe[:, 1::2] = np.cos(pos * div[:-1])
    return pe


def host_inputs(x, emb, in_proj_w, in_proj_b, out_proj_w, out_proj_b,
                ln1_g, ln1_b, ln2_g, ln2_b, ff1_w, ff1_b, ff2_w, ff2_b,
                fc_w, fc_b):
    x = np.asarray(x).astype(np.int64)
    emb = np.asarray(emb, dtype=np.float32)
    f32 = lambda a: np.ascontiguousarray(np.asarray(a, dtype=np.float32))
    in_proj_w, in_proj_b = f32(in_proj_w), f32(in_proj_b)
    out_proj_w, out_proj_b = f32(out_proj_w), f32(out_proj_b)
    ff1_w, ff1_b, ff2_w, ff2_b = f32(ff1_w), f32(ff1_b), f32(ff2_w), f32(ff2_b)
    ln1_g, ln1_b, ln2_g, ln2_b = f32(ln1_g), f32(ln1_b), f32(ln2_g), f32(ln2_b)
    fc_w, fc_b = f32(fc_w), f32(fc_b)

    h0 = emb[x] * np.float32(SQRT_D)
    h0 = h0 + _pos_encoding_np()[:, None, :]
    h0t = np.transpose(h0, (0, 2, 1))          # [B, D, S]

    def aug(wT, bias):
        return np.concatenate([wT, bias[None, :]], axis=0).astype(np.float32)

    packw = np.zeros((37, 288), np.float32)
    for l in range(L):
        packw[0:D + 1, 70 * l:70 * l + D] = aug(
            in_proj_w[l][0:D].T * SCALE, in_proj_b[l][0:D] * SCALE)
        packw[0:D + 1, 70 * l + 32:70 * l + 32 + D] = aug(
            in_proj_w[l][D:2 * D].T, in_proj_b[l][D:2 * D])
        packw[0:D + 1, 70 * l + 64:70 * l + 64 + D] = aug(
            in_proj_w[l][2 * D:3 * D].T, in_proj_b[l][2 * D:3 * D])
        packw[0:D + 1, 140 + D * l:140 + D * (l + 1)] = aug(
            out_proj_w[l].T, out_proj_b[l])
        packw[0:D + 1, 150 + F * l:150 + F * (l + 1)] = aug(
            ff1_w[l].T, ff1_b[l])
        packw[0:F + 1, 190 + D * l:190 + D * (l + 1)] = aug(
            ff2_w[l].T, ff2_b[l])
        for i, (g, bb) in enumerate(((ln1_g[l], ln1_b[l]),
                                     (ln2_g[l], ln2_b[l]))):
            k = 2 * l + i
            packw[0, 200 + D * k:200 + D * (k + 1)] = g
            packw[32, 220 + D * k:220 + D * (k + 1)] = g
            packw[0:D, 278 + k] = bb
    packw[0:D, 240] = 1.0 / D          # c2 col 0: s1 = mean(x)
    packw[32:32 + D, 272] = 1.0 / D    # c2 col 32: s2 = mean(x^2)
    packw[0:D, 273:278] = np.eye(D, dtype=np.float32)
    packw[0, 283:288] = 1.0

    # multiplicative causal mask for the diagonal 128x128 block
    idx = np.arange(128)
    maskf = (idx[None, :] >= idx[:, None]).astype(np.float32)

    fcw = np.zeros((D + 1, VP), np.float32)
    fcw[0:D, 0:V] = fc_w.T
    fcw[D, 0:V] = fc_b
    fcw = fcw.astype(ml_dtypes.bfloat16)

    ctxa0 = np.ones((D + 1, S2), np.float32)
    f1a0 = np.ones((F + 1, S2), np.float32)
    vsb0 = np.zeros((128, QC, 37), np.float32)
    vsb0[:, :, 32:37] = 1.0
    hf0 = np.ones((D + 1, S2), ml_dtypes.bfloat16)
    rr0 = np.zeros((33, S), np.float32)

    shared = dict(packw=packw, mask=np.ascontiguousarray(maskf), fcw=fcw,
                  ctxa0=ctxa0, f1a0=f1a0, vsb0=vsb0, hf0=hf0, rr0=rr0)
    in_maps = []
    for c in range(NCORES):
        hh = np.ones((D + 1, S2), np.float32)
        for b in range(BPC):
            hh[0:D, b * S:(b + 1) * S] = h0t[c * BPC + b]
        in_maps.append(dict(h0=hh, **shared))
    return in_maps


def run(in_maps, trace=False, iters=1, **kw):
    nc = _get_program(iters)
    return run_bass_kernel_spmd(nc, in_maps, list(range(NCORES)),
                                trace=trace, **kw)


def unshard(res):
    """Per-core [128, BPC, QC, VP] fp16 -> [B, S, V] fp32."""
    outs = []
    for c in range(NCORES):
        a = np.asarray(res.results[c]["out"]).astype(np.float32)
        a = a.reshape(128, BPC, QC, VP)
        a = np.transpose(a, (1, 2, 0, 3)).reshape(BPC, S, VP)[:, :, 0:V]
        outs.append(a)
    return np.ascontiguousarray(np.concatenate(outs, axis=0))


def kernel(**inputs) -> np.ndarray:
    in_maps = host_inputs(**inputs)
    res = run(in_maps)
    return unshard(res)


if __name__ == "__main__":
    import reference
    ins = {k: np.asarray(v) for k, v in reference.setup_inputs().items()}
    got = kernel(**ins)
    exp = np.asarray(reference.reference(**reference.setup_inputs()))
    err = np.abs(got - exp)
    rel = err.max() / (np.abs(exp).max() + 1e-30)
    print("max abs err:", err.max(), "rel:", rel)


# revision 50
# speedup vs baseline: 1.0132x; 1.0102x over previous
"""MicroTransformer (B=16,S=512,V=8000,D=5,F=20,L=2) on 8 trn2 NeuronCores.

Sharding: pure data parallel over batch (2 batch elements per core).
All parameters replicated. Whole transformer body + logits matmul run on
device; host only does input prep (embedding row gather, positional
encoding constant, weight layout transforms) and the final reshape.

Per-core device program (Bass/Tile, fully unrolled), v2:
  state h [6, 1024] f32r: rows 0-4 = h^T for batch0|batch1, row 5 = ones.
  qkv+vT in ONE matmul: stationary [6,69] (q scaled at cols 0-4, k at
  32-36, vT at 64-68 -> all psum reads 32-aligned).  V is transposed to
  [128k, 5] per chunk via 4 PE-transposes (identity stationary) and cast
  bf16 into the v-aug tile (ones col 32 accumulates Z).
  Attention transposed ([k,q]), softmax without row-max; exp -> bf16,
  causal mask = in-place [128,128] bf16 multiply on the diagonal block
  only (off-diagonal needs no mask).
  LayerNorm: stats via 2 matmuls into ONE psum bank (s1@row0, s2@row32,
  g*rstd@64, g*mu*rstd@96); rstd = Exp(-0.5*Ln(var+eps)) so every ACT
  function (Exp/Ln/Square/Copy/Relu-free) lives in ONE table set (no
  mid-kernel ACT table swaps).  Broadcast matmuls take a zero-initialized
  [33,512] moving tile with rstd@row0, mu*rstd@row32.
  Logits (the memory-bound phase): final h -> hfin [6,1024] bf16; fcw
  [6,8192] bf16 (vocab padded).  Flat 512-col matmuls, stationary = one
  [6,128] token chunk reused for 16 consecutive matmuls (no row-tiling,
  no fcw replicas).  Psum ring = 2x [128,1024] tiles; casts to fp16
  stage tiles ping-pong DVE/ACT; 0.5MB stores alternate the sync and
  gpsimd HWDGE rings so DMA streams continuously.  Batch 0's logits are
  emitted right after its final LN2 with batch 1's ffn tail interleaved
  mid-stream, so stores start as early as possible and never gap.
"""

import math

import numpy as np
import ml_dtypes

import concourse.bacc as bacc
import concourse.bass as bass
import concourse.mybir as mybir
import concourse.tile as tile
from concourse.bass_utils import run_bass_kernel_spmd

F32 = mybir.dt.float32
F32R = mybir.dt.float32r
BF16 = mybir.dt.bfloat16
F16 = mybir.dt.float16
ALU = mybir.AluOpType
ACTF = mybir.ActivationFunctionType


def _r(ap):
    return ap.bitcast(F32R)


def _f(ap):
    return ap.bitcast(F32)


B, S, V, D, F, L = 16, 512, 8000, 5, 20, 2
VP = 8192                      # vocab padded to 16x512
EPS = 1e-5
NCORES = 8
BPC = B // NCORES              # 2
SQRT_D = math.sqrt(float(D))
SCALE = 1.0 / SQRT_D
QC = S // 128                  # 4
S2 = BPC * S                   # 1024
NR = VP // 1024                # 8 logit rounds per (b, i)

_CACHED = {}

_ACT_SET = "natural_log_exp_and_others"


def _pin_act_tables():
    """Expose only the one table set covering Exp/Ln/Square/Copy.

    The act-table placement pass picks the first set containing each
    activation's function; with the full list it ping-pongs between the
    exp set and the ln set on every LayerNorm (measured 17 mid-kernel
    ~1.3us table loads).  Restricting the choice to the single covering
    set yields exactly one load at kernel start.
    """
    orig = bacc.get_activation_tables

    def only_covering(arch):
        t = orig(arch)
        # act_func_set_id is positional into act_info.json, so keep every
        # entry in place and just make the non-covering sets unchoosable.
        return {name: (funcs if name == _ACT_SET else set())
                for name, funcs in t.items()}

    bacc.get_activation_tables = only_covering
    return orig


def _build_program(iters=1):
    _orig_tables = _pin_act_tables()
    try:
        return _build_program_inner(iters)
    finally:
        bacc.get_activation_tables = _orig_tables


def _build_program_inner(iters=1):
    nc = bacc.Bacc("TRN2", target_bir_lowering=False, debug=False,
                   num_devices=NCORES)

    d_h0 = nc.dram_tensor("h0", [D + 1, S2], F32R, kind="ExternalInput")
    d_pack = nc.dram_tensor("packw", [37, 288], F32R, kind="ExternalInput")
    d_mask = nc.dram_tensor("mask", [128, 128], F32, kind="ExternalInput")
    d_ctxa0 = nc.dram_tensor("ctxa0", [D + 1, S2], F32R,
                             kind="ExternalInput")
    d_f1a0 = nc.dram_tensor("f1a0", [F + 1, S2], F32R, kind="ExternalInput")
    d_vsb0 = nc.dram_tensor("vsb0", [128, QC, 37], F32R,
                            kind="ExternalInput")
    d_hf0 = nc.dram_tensor("hf0", [D + 1, S2], BF16, kind="ExternalInput")
    d_rr0 = nc.dram_tensor("rr0", [33, S], F32R, kind="ExternalInput")
    d_fcw = nc.dram_tensor("fcw", [D + 1, VP], BF16, kind="ExternalInput")
    d_out = nc.dram_tensor("out", [128, BPC, QC, VP], F16,
                           kind="ExternalOutput")

    from contextlib import ExitStack
    with tile.TileContext(nc) as tc, ExitStack() as es, \
            nc.allow_low_precision(reason="bf16/f16 rounding intended"):
        cst = es.enter_context(tc.tile_pool(name="cst", bufs=1))
        wrk = es.enter_context(tc.tile_pool(name="wrk", bufs=4))
        att = es.enter_context(tc.tile_pool(name="att", bufs=4))
        stg = es.enter_context(tc.tile_pool(name="stg", bufs=6))
        # one 4-bank pool set per batch: the two body chains never share
        # psum, so they truly run in parallel; each batch's logits phase
        # then recycles pools that are idle by that point.
        ps_cz = [es.enter_context(tc.tile_pool(name=f"ps_cz{b}", bufs=1,
                                               space="PSUM"))
                 for b in range(BPC)]
        ps_ms = [es.enter_context(tc.tile_pool(name=f"ps_ms{b}", bufs=3,
                                               space="PSUM"))
                 for b in range(BPC)]

        # ---- constants ----
        h = cst.tile([D + 1, S2], F32R, name="h", tag="h")
        nc.sync.dma_start(h[:], d_h0[:])
        pack = cst.tile([37, 288], F32R, name="pack", tag="pack")
        nc.sync.dma_start(pack[:], d_pack[:])

        wqkv = [pack[0:D + 1, 70 * l:70 * l + 69] for l in range(L)]
        wo = [pack[0:D + 1, 140 + D * l:140 + D * (l + 1)] for l in range(L)]
        w1 = [pack[0:D + 1, 150 + F * l:150 + F * (l + 1)] for l in range(L)]
        w2 = [pack[0:F + 1, 190 + D * l:190 + D * (l + 1)] for l in range(L)]
        grow0 = [pack[0:33, 200 + D * k:200 + D * (k + 1)] for k in range(4)]
        grow32 = [pack[0:33, 220 + D * k:220 + D * (k + 1)] for k in range(4)]
        c02 = pack[0:D, 240:241]       # stats stationary (1/D column)
        eye5 = _f(pack[0:D, 273:278])
        bcol = [_f(pack[0:D, 278 + k:279 + k]) for k in range(4)]
        ones5 = pack[0:1, 283:288]

        maskb = cst.tile([128, 128], F32, name="maskb", tag="maskb")
        nc.gpsimd.dma_start(maskb[:], d_mask[:])
        c30 = cst.tile([128, 1], F32, name="c30", tag="c30")
        nc.vector.memset(c30[:], -30.0)
        ctxa = cst.tile([D + 1, S2], F32R, name="ctxa", tag="ctxa")
        nc.gpsimd.dma_start(ctxa[:], d_ctxa0[:])
        f1a = cst.tile([F + 1, S2], F32R, name="f1a", tag="f1a")
        nc.gpsimd.dma_start(f1a[:], d_f1a0[:])
        vsb = []
        for b in range(BPC):
            t = cst.tile([128, QC, 37], F32R, name=f"vsb{b}", tag=f"vsb{b}")
            nc.gpsimd.dma_start(t[:], d_vsb0[:])
            vsb.append(t)
        rr = []
        for b in range(BPC):
            t = cst.tile([33, S], F32R, name=f"rr{b}", tag=f"rr{b}")
            nc.gpsimd.dma_start(t[:], d_rr0[:])
            rr.append(t)
        hfin = cst.tile([D + 1, S2], BF16, name="hfin", tag="hfin")
        nc.gpsimd.dma_start(hfin[:], d_hf0[:])
        fcw = cst.tile([D + 1, VP], BF16, name="fcw", tag="fcw")
        nc.gpsimd.dma_start(fcw[:], d_fcw[:])

        def layernorm(l, i, b, it, resid_ap, add_ps, out_ap):
            """out = LN(resid + add_ps) * g + b.

            Two independent 256-token half-chains; halving the free size
            halves every op's duration, and the halves pipeline across
            engines, roughly halving the ~13-step serial latency.
            """
            u = f"{l}{i}{b}_{it}"
            k = 2 * l + i
            HS = S // 2
            x = wrk.tile([D, S], F32R, name=f"lx{u}", tag="lx")
            xsq = wrk.tile([D, S], F32R, name=f"lq{u}", tag="lq")
            s1 = ps_ms[b].tile([1, S], F32, name=f"s1{u}", tag="ps_ms")
            s2 = ps_ms[b].tile([1, S], F32, name=f"s2{u}", tag="ps_ms")
            t1 = wrk.tile([1, S], F32, name=f"t1{u}", tag="lt1")
            var = wrk.tile([1, S], F32, name=f"lv{u}", tag="lvar")
            lnv = wrk.tile([1, S], F32, name=f"ll{u}", tag="llnv")
            rbp = ps_ms[b].tile([D, S], F32, name=f"lr{u}", tag="ps_ms")
            rmp = ps_ms[b].tile([D, S], F32, name=f"lm{u}", tag="ps_ms")
            t2 = wrk.tile([D, S], F32, name=f"t2{u}", tag="lt2")
            hs = [slice(0, HS), slice(HS, S)]
            for hh in hs:
                nc.vector.tensor_add(x[:, hh], resid_ap[:, hh],
                                     add_ps[:, hh])
            for hh in hs:
                nc.gpsimd.tensor_mul(xsq[:, hh], x[:, hh], x[:, hh])
                nc.tensor.matmul(s1[:, hh], c02, x[:, hh])
            for hh in hs:
                nc.tensor.matmul(s2[:, hh], c02, xsq[:, hh])
                nc.scalar.square(t1[:, hh], s1[:, hh])
            for hh in hs:
                nc.vector.scalar_tensor_tensor(var[:, hh], s2[:, hh], EPS,
                                               t1[:, hh],
                                               op0=ALU.add, op1=ALU.subtract)
            for hh in hs:
                nc.scalar.activation(lnv[:, hh], var[:, hh], ACTF.Ln)
            for hh in hs:
                nc.scalar.activation(rr[b][0:1, hh], lnv[:, hh], ACTF.Exp,
                                     scale=-0.5)
                nc.vector.tensor_mul(rr[b][32:33, hh], _f(rr[b][0:1, hh]),
                                     s1[:, hh])
            for hh in hs:
                nc.tensor.matmul(rbp[:, hh], grow0[k], rr[b][:, hh])
                nc.tensor.matmul(rmp[:, hh], grow32[k], rr[b][:, hh])
            for hh in hs:
                nc.vector.tensor_mul(t2[:, hh], _f(x[:, hh]), rbp[:, hh])
            for hh in hs:
                nc.vector.scalar_tensor_tensor(out_ap[:, hh], t2[:, hh],
                                               bcol[k], rmp[:, hh],
                                               op0=ALU.add, op1=ALU.subtract)

        def attn(l, b, it):
            u = f"{l}{b}_{it}"
            hb = h[:, b * S:(b + 1) * S]
            qkp = ps_ms[b].tile([69, S], F32, name=f"qk{u}", tag="ps_ms")
            nc.tensor.matmul(qkp[:], wqkv[l], hb)
            qsb = wrk.tile([D, S], F32R, name=f"qs{u}", tag="qsb")
            ksb = wrk.tile([D, S], F32R, name=f"ks{u}", tag="ksb")
            vTs = wrk.tile([D, S], F32, name=f"vt{u}", tag="vts")
            nc.scalar.copy(qsb[:], qkp[0:D, :])
            nc.vector.tensor_copy(ksb[:], qkp[32:32 + D, :])
            # scores first (chain-critical); the v path fills engine gaps
            sc_tiles = []
            expTs = []
            for kc in range(QC):
                off = kc * 128
                n = S - off
                scp = ps_ms[b].tile([128, S], F32, name=f"sc{u}{kc}",
                                    tag="ps_ms")
                nc.tensor.matmul(scp[:, 0:n], ksb[:, off:off + 128],
                                 qsb[:, off:S])
                sc_tiles.append(scp)
                if kc == 0:
                    nc.scalar.copy(vTs[:], qkp[64:64 + D, :])
            expQ = []
            vtp = ps_ms[b].tile([128, QC, 8], F32, name=f"vp{u}", tag="ps_ms")
            for kc in range(QC):
                off = kc * 128
                n = S - off
                scp = sc_tiles[kc]
                expT = att.tile([128, S], F32R, name=f"ex{u}{kc}", tag="expT")
                # softmax is shift-invariant: exp(s-30) keeps the table's
                # input/output range comfortable (scores reach ~|50|)
                nc.scalar.activation(expT[:, 0:n], scp[:, 0:n], ACTF.Exp,
                                     bias=c30[:])
                nc.gpsimd.tensor_mul(expT[:, 0:128], expT[:, 0:128],
                                     maskb[:])
                expQ.append(expT)
                if kc == 0:
                    for t in range(QC):
                        nc.tensor.transpose(vtp[:, t, 0:D],
                                            vTs[:, t * 128:(t + 1) * 128],
                                            eye5)
                    nc.vector.tensor_copy(vsb[b][:, :, 0:D], vtp[:, :, 0:D])
            cz = ps_cz[b].tile([37, S], F32, name=f"cz{u}", tag="ps_cz")
            for kc in range(QC):
                off = kc * 128
                n = S - off
                nc.tensor.matmul(cz[:, off:S], vsb[b][:, kc, :],
                                 expQ[kc][:, 0:n],
                                 start=(kc == 0), stop=(kc == QC - 1))
            zf = wrk.tile([D, S], F32, name=f"zf{u}", tag="zf")
            nc.scalar.copy(zf[:], cz[32:32 + D, :])
            ctxs = wrk.tile([D, S], F32, name=f"cs{u}", tag="ctxs")
            nc.vector.tensor_copy(ctxs[:], cz[0:D, :])
            zr = wrk.tile([D, S], F32, name=f"zr{u}", tag="zr")
            nc.vector.reciprocal_approx_fast(zr[:], zf[:])
            nc.gpsimd.tensor_mul(ctxa[0:D, b * S:(b + 1) * S], ctxs[:],
                                 zr[:])

        def proj_ln1(l, b, it):
            bc0 = b * S
            pp = ps_ms[b].tile([D, S], F32, name=f"pp{l}{b}_{it}", tag="ps_ms")
            nc.tensor.matmul(pp[:], wo[l], ctxa[:, bc0:bc0 + S])
            layernorm(l, 0, b, it, h[0:D, bc0:bc0 + S], pp[:],
                      h[0:D, bc0:bc0 + S])

        def ffn_ln2(l, b, it):
            bc0 = b * S
            f1p = ps_ms[b].tile([F, S], F32, name=f"f1{l}{b}_{it}", tag="ps_ms")
            nc.tensor.matmul(f1p[:], w1[l], h[:, bc0:bc0 + S])
            nc.vector.tensor_scalar_max(f1a[0:F, bc0:bc0 + S // 2],
                                        f1p[:, 0:S // 2], 0.0)
            nc.vector.tensor_scalar_max(f1a[0:F, bc0 + S // 2:bc0 + S],
                                        f1p[:, S // 2:S], 0.0)
            f2p = ps_ms[b].tile([D, S], F32, name=f"f2{l}{b}_{it}", tag="ps_ms")
            nc.tensor.matmul(f2p[:], w2[l], f1a[:, bc0:bc0 + S])
            if l == L - 1:
                layernorm(l, 1, b, it, h[0:D, bc0:bc0 + S], f2p[:],
                          hfin[0:D, bc0:bc0 + S])
            else:
                layernorm(l, 1, b, it, h[0:D, bc0:bc0 + S], f2p[:],
                          h[0:D, bc0:bc0 + S])

        def logits_b(b, it, tails=None):
            """32 rounds of [2 matmuls -> cast -> staged 0.5MB store].

            tails: {round_index: callback} -- emits the other batch's body
            segments between rounds so its work drains during this batch's
            store stream (engine queues are in-order; segments must be
            small enough not to starve the cast/store pipeline).
            """
            # this batch's own body pools are free once hfin is ready;
            # batch 1's logits can additionally recycle batch 0's pools
            # (its logits phase has completed by then) for a deeper ring.
            ring = [(ps_cz[b], "ps_cz"), (ps_ms[b], "ps_ms"),
                    (ps_ms[b], "ps_ms"), (ps_ms[b], "ps_ms"),
                    (ps_cz[1 - b], "ps_cz")]
            if b == 1:
                ring = ring + [(ps_cz[0], "ps_cz"), (ps_ms[0], "ps_ms"),
                               (ps_ms[0], "ps_ms"), (ps_ms[0], "ps_ms")]
            nr = 0
            st = None
            for i in range(QC):
                stat = hfin[0:D + 1, b * S + 128 * i:b * S + 128 * (i + 1)]
                for r in range(VP // 512):
                    pool, tg = ring[nr % len(ring)]
                    lp = pool.tile([128, 512], F32,
                                   name=f"lp{b}{i}{r}_{it}", tag=tg)
                    v0 = r * 512
                    nc.tensor.matmul(lp[:], stat, fcw[:, v0:v0 + 512])
                    if r % 4 == 0:
                        st = stg.tile([128, 2048], F16,
                                      name=f"st{b}{i}{r}_{it}", tag="stage")
                    sl = st[:, (r % 4) * 512:(r % 4) * 512 + 512]
                    if nr % 2 == 0:
                        nc.vector.tensor_copy(sl, lp[:])
                    else:
                        nc.scalar.copy(sl, lp[:])
                    if r % 4 == 3:
                        eng = nc.sync if (nr // 4) % 2 == 0 else nc.gpsimd
                        eng.dma_start(d_out[:, b, i, v0 - 1536:v0 + 512],
                                      st[:])
                    nr += 1
                    if tails and nr in tails:
                        tails[nr]()

        for it in range(iters):
            if it > 0:
                nc.sync.dma_start(h[:], d_h0[:])
            # stepwise batch interleave: the two chains fill each other's
            # engine gaps, and both logits phases then run contention-free
            # (pure matmul+cast+store streams).
            for l in range(L):
                for b in range(BPC):
                    attn(l, b, it)
                for b in range(BPC):
                    proj_ln1(l, b, it)
                if l < L - 1:
                    for b in range(BPC):
                        ffn_ln2(l, b, it)
            # final-layer tail: emit batch 0's ffn+LN2, then its logits with
            # batch 1's ffn+LN2 injected mid-stream so batch 1's hfin is
            # ready just before its own logits rounds begin.
            for b in range(BPC):
                ffn_ln2(L - 1, b, it)
            logits_b(0, it)
            logits_b(1, it)

    nc.compile()
    return nc


def _get_program(iters=1):
    if iters not in _CACHED:
        _CACHED[iters] = _build_program(iters)
    return _CACHED[iters]


def _pos_encoding_np():
    pos = np.arange(B, dtype=np.float32)[:, None]
    div = np.exp(np.arange(0, D, 2, dtype=np.float32)
                 * (-math.log(10000.0) / D))
    pe = np.zeros((B, D), dtype=np.float32)
    pe[:, 0::2] = np.sin(pos * div)
    pe[:, 1::2] = np.cos(pos * div[:-1])
    return pe


def host_inputs(x, emb, in_proj_w, in_proj_b, out_proj_w, out_proj_b,
                ln1_g, ln1_b, ln2_g, ln2_b, ff1_w, ff1_b, ff2_w, ff2_b,
                fc_w, fc_b):
    x = np.asarray(x).astype(np.int64)
    emb = np.asarray(emb, dtype=np.float32)
    f32 = lambda a: np.ascontiguousarray(np.asarray(a, dtype=np.float32))
    in_proj_w, in_proj_b = f32(in_proj_w), f32(in_proj_b)
    out_proj_w, out_proj_b = f32(out_proj_w), f32(out_proj_b)
    ff1_w, ff1_b, ff2_w, ff2_b = f32(ff1_w), f32(ff1_b), f32(ff2_w), f32(ff2_b)
    ln1_g, ln1_b, ln2_g, ln2_b = f32(ln1_g), f32(ln1_b), f32(ln2_g), f32(ln2_b)
    fc_w, fc_b = f32(fc_w), f32(fc_b)

    h0 = emb[x] * np.float32(SQRT_D)
    h0 = h0 + _pos_encoding_np()[:, None, :]
    h0t = np.transpose(h0, (0, 2, 1))          # [B, D, S]

    def aug(wT, bias):
        return np.concatenate([wT, bias[None, :]], axis=0).astype(np.float32)

    packw = np.zeros((37, 288), np.float32)
    for l in range(L):
        packw[0:D + 1, 70 * l:70 * l + D] = aug(
            in_proj_w[l][0:D].T * SCALE, in_proj_b[l][0:D] * SCALE)
        packw[0:D + 1, 70 * l + 32:70 * l + 32 + D] = aug(
            in_proj_w[l][D:2 * D].T, in_proj_b[l][D:2 * D])
        packw[0:D + 1, 70 * l + 64:70 * l + 64 + D] = aug(
            in_proj_w[l][2 * D:3 * D].T, in_proj_b[l][2 * D:3 * D])
        packw[0:D + 1, 140 + D * l:140 + D * (l + 1)] = aug(
            out_proj_w[l].T, out_proj_b[l])
        packw[0:D + 1, 150 + F * l:150 + F * (l + 1)] = aug(
            ff1_w[l].T, ff1_b[l])
        packw[0:F + 1, 190 + D * l:190 + D * (l + 1)] = aug(
            ff2_w[l].T, ff2_b[l])
        for i, (g, bb) in enumerate(((ln1_g[l], ln1_b[l]),
                                     (ln2_g[l], ln2_b[l]))):
            k = 2 * l + i
            packw[0, 200 + D * k:200 + D * (k + 1)] = g
            packw[32, 220 + D * k:220 + D * (k + 1)] = g
            packw[0:D, 278 + k] = bb
    packw[0:D, 240] = 1.0 / D          # c2 col 0: s1 = mean(x)
    packw[32:32 + D, 272] = 1.0 / D    # c2 col 32: s2 = mean(x^2)
    packw[0:D, 273:278] = np.eye(D, dtype=np.float32)
    packw[0, 283:288] = 1.0

    # multiplicative causal mask for the diagonal 128x128 block
    idx = np.arange(128)
    maskf = (idx[None, :] >= idx[:, None]).astype(np.float32)

    fcw = np.zeros((D + 1, VP), np.float32)
    fcw[0:D, 0:V] = fc_w.T
    fcw[D, 0:V] = fc_b
    fcw = fcw.astype(ml_dtypes.bfloat16)

    ctxa0 = np.ones((D + 1, S2), np.float32)
    f1a0 = np.ones((F + 1, S2), np.float32)
    vsb0 = np.zeros((128, QC, 37), np.float32)
    vsb0[:, :, 32:37] = 1.0
    hf0 = np.ones((D + 1, S2), ml_dtypes.bfloat16)
    rr0 = np.zeros((33, S), np.float32)

    shared = dict(packw=packw, mask=np.ascontiguousarray(maskf), fcw=fcw,
                  ctxa0=ctxa0, f1a0=f1a0, vsb0=vsb0, hf0=hf0, rr0=rr0)
    in_maps = []
    for c in range(NCORES):
        hh = np.ones((D + 1, S2), np.float32)
        for b in range(BPC):
            hh[0:D, b * S:(b + 1) * S] = h0t[c * BPC + b]
        in_maps.append(dict(h0=hh, **shared))
    return in_maps


def run(in_maps, trace=False, iters=1, **kw):
    nc = _get_program(iters)
    return run_bass_kernel_spmd(nc, in_maps, list(range(NCORES)),
                                trace=trace, **kw)


def unshard(res):
    """Per-core [128, BPC, QC, VP] fp16 -> [B, S, V] fp32."""
    outs = []
    for c in range(NCORES):
        a = np.asarray(res.results[c]["out"]).astype(np.float32)
        a = a.reshape(128, BPC, QC, VP)
        a = np.transpose(a, (1, 2, 0, 3)).reshape(BPC, S, VP)[:, :, 0:V]
        outs.append(a)
    return np.ascontiguousarray(np.concatenate(outs, axis=0))


def kernel(**inputs) -> np.ndarray:
    in_maps = host_inputs(**inputs)
    res = run(in_maps)
    return unshard(res)


if __name__ == "__main__":
    import reference
    ins = {k: np.asarray(v) for k, v in reference.setup_inputs().items()}
    got = kernel(**ins)
    exp = np.asarray(reference.reference(**reference.setup_inputs()))
    err = np.abs(got - exp)
    rel = err.max() / (np.abs(exp).max() + 1e-30)
    print("max abs err:", err.max(), "rel:", rel)


# revision 51
# speedup vs baseline: 1.0433x; 1.0297x over previous
"""MicroTransformer (B=16,S=512,V=8000,D=5,F=20,L=2) on 8 trn2 NeuronCores.

Sharding: pure data parallel over batch (2 batch elements per core).
All parameters replicated. Whole transformer body + logits matmul run on
device; host only does input prep (embedding row gather, positional
encoding constant, weight layout transforms) and the final reshape.

Per-core device program (Bass/Tile, fully unrolled), v2:
  state h [6, 1024] f32r: rows 0-4 = h^T for batch0|batch1, row 5 = ones.
  qkv+vT in ONE matmul: stationary [6,69] (q scaled at cols 0-4, k at
  32-36, vT at 64-68 -> all psum reads 32-aligned).  V is transposed to
  [128k, 5] per chunk via 4 PE-transposes (identity stationary) and cast
  bf16 into the v-aug tile (ones col 32 accumulates Z).
  Attention transposed ([k,q]), softmax without row-max; exp -> bf16,
  causal mask = in-place [128,128] bf16 multiply on the diagonal block
  only (off-diagonal needs no mask).
  LayerNorm: stats via 2 matmuls into ONE psum bank (s1@row0, s2@row32,
  g*rstd@64, g*mu*rstd@96); rstd = Exp(-0.5*Ln(var+eps)) so every ACT
  function (Exp/Ln/Square/Copy/Relu-free) lives in ONE table set (no
  mid-kernel ACT table swaps).  Broadcast matmuls take a zero-initialized
  [33,512] moving tile with rstd@row0, mu*rstd@row32.
  Logits (the memory-bound phase): final h -> hfin [6,1024] bf16; fcw
  [6,8192] bf16 (vocab padded).  Flat 512-col matmuls, stationary = one
  [6,128] token chunk reused for 16 consecutive matmuls (no row-tiling,
  no fcw replicas).  Psum ring = 2x [128,1024] tiles; casts to fp16
  stage tiles ping-pong DVE/ACT; 0.5MB stores alternate the sync and
  gpsimd HWDGE rings so DMA streams continuously.  Batch 0's logits are
  emitted right after its final LN2 with batch 1's ffn tail interleaved
  mid-stream, so stores start as early as possible and never gap.
"""

import math

import numpy as np
import ml_dtypes

import concourse.bacc as bacc
import concourse.bass as bass
import concourse.mybir as mybir
import concourse.tile as tile
from concourse.bass_utils import run_bass_kernel_spmd

F32 = mybir.dt.float32
F32R = mybir.dt.float32r
BF16 = mybir.dt.bfloat16
F16 = mybir.dt.float16
ALU = mybir.AluOpType
ACTF = mybir.ActivationFunctionType


def _r(ap):
    return ap.bitcast(F32R)


def _f(ap):
    return ap.bitcast(F32)


B, S, V, D, F, L = 16, 512, 8000, 5, 20, 2
VP = 8192                      # vocab padded to 16x512
EPS = 1e-5
NCORES = 8
BPC = B // NCORES              # 2
SQRT_D = math.sqrt(float(D))
SCALE = 1.0 / SQRT_D
QC = S // 128                  # 4
S2 = BPC * S                   # 1024
NR = VP // 1024                # 8 logit rounds per (b, i)

_CACHED = {}

_ACT_SET = "natural_log_exp_and_others"


def _pin_act_tables():
    """Expose only the one table set covering Exp/Ln/Square/Copy.

    The act-table placement pass picks the first set containing each
    activation's function; with the full list it ping-pongs between the
    exp set and the ln set on every LayerNorm (measured 17 mid-kernel
    ~1.3us table loads).  Restricting the choice to the single covering
    set yields exactly one load at kernel start.
    """
    orig = bacc.get_activation_tables

    def only_covering(arch):
        t = orig(arch)
        # act_func_set_id is positional into act_info.json, so keep every
        # entry in place and just make the non-covering sets unchoosable.
        return {name: (funcs if name == _ACT_SET else set())
                for name, funcs in t.items()}

    bacc.get_activation_tables = only_covering
    return orig


def _build_program(iters=1):
    _orig_tables = _pin_act_tables()
    try:
        return _build_program_inner(iters)
    finally:
        bacc.get_activation_tables = _orig_tables


def _build_program_inner(iters=1):
    nc = bacc.Bacc("TRN2", target_bir_lowering=False, debug=False,
                   num_devices=NCORES)

    d_h0 = nc.dram_tensor("h0", [D + 1, S2], F32R, kind="ExternalInput")
    d_pack = nc.dram_tensor("packw", [37, 288], F32R, kind="ExternalInput")
    d_mask = nc.dram_tensor("mask", [128, 128], F32, kind="ExternalInput")
    d_ctxa0 = nc.dram_tensor("ctxa0", [D + 1, S2], F32R,
                             kind="ExternalInput")
    d_f1a0 = nc.dram_tensor("f1a0", [F + 1, S2], F32R, kind="ExternalInput")
    d_vsb0 = nc.dram_tensor("vsb0", [128, QC, 37], F32R,
                            kind="ExternalInput")
    d_hf0 = nc.dram_tensor("hf0", [D + 1, S2], BF16, kind="ExternalInput")
    d_rr0 = nc.dram_tensor("rr0", [33, S], F32R, kind="ExternalInput")
    d_fcw = nc.dram_tensor("fcw", [D + 1, VP], BF16, kind="ExternalInput")
    d_out = nc.dram_tensor("out", [128, BPC, QC, VP], F16,
                           kind="ExternalOutput")

    from contextlib import ExitStack
    with tile.TileContext(nc) as tc, ExitStack() as es, \
            nc.allow_low_precision(reason="bf16/f16 rounding intended"):
        cst = es.enter_context(tc.tile_pool(name="cst", bufs=1))
        wrk = es.enter_context(tc.tile_pool(name="wrk", bufs=4))
        att = es.enter_context(tc.tile_pool(name="att", bufs=4))
        stg = es.enter_context(tc.tile_pool(name="stg", bufs=6))
        # one 4-bank pool set per batch: the two body chains never share
        # psum, so they truly run in parallel; each batch's logits phase
        # then recycles pools that are idle by that point.
        ps_cz = [es.enter_context(tc.tile_pool(name=f"ps_cz{b}", bufs=1,
                                               space="PSUM"))
                 for b in range(BPC)]
        ps_ms = [es.enter_context(tc.tile_pool(name=f"ps_ms{b}", bufs=3,
                                               space="PSUM"))
                 for b in range(BPC)]

        # ---- constants ----
        h = cst.tile([D + 1, S2], F32R, name="h", tag="h")
        nc.sync.dma_start(h[:], d_h0[:])
        pack = cst.tile([37, 288], F32R, name="pack", tag="pack")
        nc.sync.dma_start(pack[:], d_pack[:])

        wqkv = [pack[0:D + 1, 70 * l:70 * l + 69] for l in range(L)]
        wo = [pack[0:D + 1, 140 + D * l:140 + D * (l + 1)] for l in range(L)]
        w1 = [pack[0:D + 1, 150 + F * l:150 + F * (l + 1)] for l in range(L)]
        w2 = [pack[0:F + 1, 190 + D * l:190 + D * (l + 1)] for l in range(L)]
        grow0 = [pack[0:33, 200 + D * k:200 + D * (k + 1)] for k in range(4)]
        grow32 = [pack[0:33, 220 + D * k:220 + D * (k + 1)] for k in range(4)]
        c02 = pack[0:D, 240:241]       # stats stationary (1/D column)
        eye5 = _f(pack[0:D, 273:278])
        bcol = [_f(pack[0:D, 278 + k:279 + k]) for k in range(4)]
        ones5 = pack[0:1, 283:288]

        maskb = cst.tile([128, 128], F32, name="maskb", tag="maskb")
        nc.gpsimd.dma_start(maskb[:], d_mask[:])
        c30 = cst.tile([128, 1], F32, name="c30", tag="c30")
        nc.vector.memset(c30[:], -30.0)
        ctxa = cst.tile([D + 1, S2], F32R, name="ctxa", tag="ctxa")
        nc.gpsimd.dma_start(ctxa[:], d_ctxa0[:])
        f1a = cst.tile([F + 1, S2], F32R, name="f1a", tag="f1a")
        nc.gpsimd.dma_start(f1a[:], d_f1a0[:])
        vsb = []
        for b in range(BPC):
            t = cst.tile([128, QC, 37], F32R, name=f"vsb{b}", tag=f"vsb{b}")
            nc.gpsimd.dma_start(t[:], d_vsb0[:])
            vsb.append(t)
        rr = []
        for b in range(BPC):
            t = cst.tile([33, S], F32R, name=f"rr{b}", tag=f"rr{b}")
            nc.gpsimd.dma_start(t[:], d_rr0[:])
            rr.append(t)
        hfin = cst.tile([D + 1, S2], BF16, name="hfin", tag="hfin")
        nc.gpsimd.dma_start(hfin[:], d_hf0[:])
        fcw = cst.tile([D + 1, VP], BF16, name="fcw", tag="fcw")
        nc.gpsimd.dma_start(fcw[:], d_fcw[:])

        def layernorm(l, i, b, it, resid_ap, add_ps, out_ap):
            """out = LN(resid + add_ps) * g + b.

            Two independent 256-token half-chains; halving the free size
            halves every op's duration, and the halves pipeline across
            engines, roughly halving the ~13-step serial latency.
            """
            u = f"{l}{i}{b}_{it}"
            k = 2 * l + i
            HS = S // 2
            x = wrk.tile([D, S], F32R, name=f"lx{u}", tag="lx")
            xsq = wrk.tile([D, S], F32R, name=f"lq{u}", tag="lq")
            s1 = ps_ms[b].tile([1, S], F32, name=f"s1{u}", tag="ps_ms")
            s2 = ps_ms[b].tile([1, S], F32, name=f"s2{u}", tag="ps_ms")
            t1 = wrk.tile([1, S], F32, name=f"t1{u}", tag="lt1")
            var = wrk.tile([1, S], F32, name=f"lv{u}", tag="lvar")
            lnv = wrk.tile([1, S], F32, name=f"ll{u}", tag="llnv")
            rbp = ps_ms[b].tile([D, S], F32, name=f"lr{u}", tag="ps_ms")
            rmp = ps_ms[b].tile([D, S], F32, name=f"lm{u}", tag="ps_ms")
            t2 = wrk.tile([D, S], F32, name=f"t2{u}", tag="lt2")
            hs = [slice(0, S)]
            for hh in hs:
                nc.vector.tensor_add(x[:, hh], resid_ap[:, hh],
                                     add_ps[:, hh])
            for hh in hs:
                nc.gpsimd.tensor_mul(xsq[:, hh], x[:, hh], x[:, hh])
                nc.tensor.matmul(s1[:, hh], c02, x[:, hh])
            for hh in hs:
                nc.tensor.matmul(s2[:, hh], c02, xsq[:, hh])
                nc.scalar.square(t1[:, hh], s1[:, hh])
            for hh in hs:
                nc.vector.scalar_tensor_tensor(var[:, hh], s2[:, hh], EPS,
                                               t1[:, hh],
                                               op0=ALU.add, op1=ALU.subtract)
            for hh in hs:
                nc.scalar.activation(lnv[:, hh], var[:, hh], ACTF.Ln)
            for hh in hs:
                nc.scalar.activation(rr[b][0:1, hh], lnv[:, hh], ACTF.Exp,
                                     scale=-0.5)
                nc.vector.tensor_mul(rr[b][32:33, hh], _f(rr[b][0:1, hh]),
                                     s1[:, hh])
            for hh in hs:
                nc.tensor.matmul(rbp[:, hh], grow0[k], rr[b][:, hh])
                nc.tensor.matmul(rmp[:, hh], grow32[k], rr[b][:, hh])
            for hh in hs:
                nc.vector.tensor_mul(t2[:, hh], _f(x[:, hh]), rbp[:, hh])
            for hh in hs:
                nc.vector.scalar_tensor_tensor(out_ap[:, hh], t2[:, hh],
                                               bcol[k], rmp[:, hh],
                                               op0=ALU.add, op1=ALU.subtract)

        def attn(l, b, it):
            u = f"{l}{b}_{it}"
            hb = h[:, b * S:(b + 1) * S]
            qkp = ps_ms[b].tile([69, S], F32, name=f"qk{u}", tag="ps_ms")
            nc.tensor.matmul(qkp[:], wqkv[l], hb)
            qsb = wrk.tile([D, S], F32R, name=f"qs{u}", tag="qsb")
            ksb = wrk.tile([D, S], F32R, name=f"ks{u}", tag="ksb")
            vTs = wrk.tile([D, S], F32, name=f"vt{u}", tag="vts")
            nc.scalar.copy(qsb[:], qkp[0:D, :])
            nc.vector.tensor_copy(ksb[:], qkp[32:32 + D, :])
            # scores first (chain-critical); the v path fills engine gaps
            sc_tiles = []
            expTs = []
            for kc in range(QC):
                off = kc * 128
                n = S - off
                scp = ps_ms[b].tile([128, S], F32, name=f"sc{u}{kc}",
                                    tag="ps_ms")
                nc.tensor.matmul(scp[:, 0:n], ksb[:, off:off + 128],
                                 qsb[:, off:S])
                sc_tiles.append(scp)
                if kc == 0:
                    nc.scalar.copy(vTs[:], qkp[64:64 + D, :])
            expQ = []
            vtp = ps_ms[b].tile([128, QC, 8], F32, name=f"vp{u}", tag="ps_ms")
            for kc in range(QC):
                off = kc * 128
                n = S - off
                scp = sc_tiles[kc]
                expT = att.tile([128, S], F32R, name=f"ex{u}{kc}", tag="expT")
                # softmax is shift-invariant: exp(s-30) keeps the table's
                # input/output range comfortable (scores reach ~|50|)
                nc.scalar.activation(expT[:, 0:n], scp[:, 0:n], ACTF.Exp,
                                     bias=c30[:])
                nc.gpsimd.tensor_mul(expT[:, 0:128], expT[:, 0:128],
                                     maskb[:])
                expQ.append(expT)
                if kc == 0:
                    for t in range(QC):
                        nc.tensor.transpose(vtp[:, t, 0:D],
                                            vTs[:, t * 128:(t + 1) * 128],
                                            eye5)
                    nc.vector.tensor_copy(vsb[b][:, :, 0:D], vtp[:, :, 0:D])
            cz = ps_cz[b].tile([37, S], F32, name=f"cz{u}", tag="ps_cz")
            for kc in range(QC):
                off = kc * 128
                n = S - off
                nc.tensor.matmul(cz[:, off:S], vsb[b][:, kc, :],
                                 expQ[kc][:, 0:n],
                                 start=(kc == 0), stop=(kc == QC - 1))
            zf = wrk.tile([D, S], F32, name=f"zf{u}", tag="zf")
            nc.scalar.copy(zf[:], cz[32:32 + D, :])
            ctxs = wrk.tile([D, S], F32, name=f"cs{u}", tag="ctxs")
            nc.vector.tensor_copy(ctxs[:], cz[0:D, :])
            zr = wrk.tile([D, S], F32, name=f"zr{u}", tag="zr")
            nc.vector.reciprocal_approx_fast(zr[:], zf[:])
            nc.gpsimd.tensor_mul(ctxa[0:D, b * S:(b + 1) * S], ctxs[:],
                                 zr[:])

        def proj_ln1(l, b, it):
            bc0 = b * S
            pp = ps_ms[b].tile([D, S], F32, name=f"pp{l}{b}_{it}", tag="ps_ms")
            nc.tensor.matmul(pp[:], wo[l], ctxa[:, bc0:bc0 + S])
            layernorm(l, 0, b, it, h[0:D, bc0:bc0 + S], pp[:],
                      h[0:D, bc0:bc0 + S])

        def ffn_ln2(l, b, it):
            bc0 = b * S
            f1p = ps_ms[b].tile([F, S], F32, name=f"f1{l}{b}_{it}", tag="ps_ms")
            nc.tensor.matmul(f1p[:], w1[l], h[:, bc0:bc0 + S])
            nc.vector.tensor_scalar_max(f1a[0:F, bc0:bc0 + S // 2],
                                        f1p[:, 0:S // 2], 0.0)
            nc.vector.tensor_scalar_max(f1a[0:F, bc0 + S // 2:bc0 + S],
                                        f1p[:, S // 2:S], 0.0)
            f2p = ps_ms[b].tile([D, S], F32, name=f"f2{l}{b}_{it}", tag="ps_ms")
            nc.tensor.matmul(f2p[:], w2[l], f1a[:, bc0:bc0 + S])
            if l == L - 1:
                layernorm(l, 1, b, it, h[0:D, bc0:bc0 + S], f2p[:],
                          hfin[0:D, bc0:bc0 + S])
            else:
                layernorm(l, 1, b, it, h[0:D, bc0:bc0 + S], f2p[:],
                          h[0:D, bc0:bc0 + S])

        def logits_b(b, it, tails=None):
            """32 rounds of [2 matmuls -> cast -> staged 0.5MB store].

            tails: {round_index: callback} -- emits the other batch's body
            segments between rounds so its work drains during this batch's
            store stream (engine queues are in-order; segments must be
            small enough not to starve the cast/store pipeline).
            """
            # this batch's own body pools are free once hfin is ready;
            # batch 1's logits can additionally recycle batch 0's pools
            # (its logits phase has completed by then) for a deeper ring.
            ring = [(ps_cz[b], "ps_cz"), (ps_ms[b], "ps_ms"),
                    (ps_ms[b], "ps_ms"), (ps_ms[b], "ps_ms"),
                    (ps_cz[1 - b], "ps_cz")]
            if b == 1:
                ring = ring + [(ps_cz[0], "ps_cz"), (ps_ms[0], "ps_ms"),
                               (ps_ms[0], "ps_ms"), (ps_ms[0], "ps_ms")]
            nr = 0
            st = None
            for i in range(QC):
                stat = hfin[0:D + 1, b * S + 128 * i:b * S + 128 * (i + 1)]
                for r in range(VP // 512):
                    pool, tg = ring[nr % len(ring)]
                    lp = pool.tile([128, 512], F32,
                                   name=f"lp{b}{i}{r}_{it}", tag=tg)
                    v0 = r * 512
                    nc.tensor.matmul(lp[:], stat, fcw[:, v0:v0 + 512])
                    if r % 4 == 0:
                        st = stg.tile([128, 2048], F16,
                                      name=f"st{b}{i}{r}_{it}", tag="stage")
                    sl = st[:, (r % 4) * 512:(r % 4) * 512 + 512]
                    if nr % 2 == 0:
                        nc.vector.tensor_copy(sl, lp[:])
                    else:
                        nc.scalar.copy(sl, lp[:])
                    if r % 4 == 3:
                        eng = nc.sync if (nr // 4) % 2 == 0 else nc.gpsimd
                        eng.dma_start(d_out[:, b, i, v0 - 1536:v0 + 512],
                                      st[:])
                    nr += 1
                    if tails and nr in tails:
                        tails[nr]()

        for it in range(iters):
            if it > 0:
                nc.sync.dma_start(h[:], d_h0[:])
            # stepwise batch interleave: the two chains fill each other's
            # engine gaps, and both logits phases then run contention-free
            # (pure matmul+cast+store streams).
            for l in range(L):
                for b in range(BPC):
                    attn(l, b, it)
                for b in range(BPC):
                    proj_ln1(l, b, it)
                if l < L - 1:
                    for b in range(BPC):
                        ffn_ln2(l, b, it)
            # final-layer tail: emit batch 0's ffn+LN2, then its logits with
            # batch 1's ffn+LN2 injected mid-stream so batch 1's hfin is
            # ready just before its own logits rounds begin.
            for b in range(BPC):
                ffn_ln2(L - 1, b, it)
            logits_b(0, it)
            logits_b(1, it)

    nc.compile()
    return nc


def _get_program(iters=1):
    if iters not in _CACHED:
        _CACHED[iters] = _build_program(iters)
    return _CACHED[iters]


def _pos_encoding_np():
    pos = np.arange(B, dtype=np.float32)[:, None]
    div = np.exp(np.arange(0, D, 2, dtype=np.float32)
                 * (-math.log(10000.0) / D))
    pe = np.zeros((B, D), dtype=np.float32)
    pe[:, 0::2] = np.sin(pos * div)
    pe[:, 1::2] = np.cos(pos * div[:-1])
    return pe


def host_inputs(x, emb, in_proj_w, in_proj_b, out_proj_w, out_proj_b,
                ln1_g, ln1_b, ln2_g, ln2_b, ff1_w, ff1_b, ff2_w, ff2_b,
                fc_w, fc_b):
    x = np.asarray(x).astype(np.int64)
    emb = np.asarray(emb, dtype=np.float32)
    f32 = lambda a: np.ascontiguousarray(np.asarray(a, dtype=np.float32))
    in_proj_w, in_proj_b = f32(in_proj_w), f32(in_proj_b)
    out_proj_w, out_proj_b = f32(out_proj_w), f32(out_proj_b)
    ff1_w, ff1_b, ff2_w, ff2_b = f32(ff1_w), f32(ff1_b), f32(ff2_w), f32(ff2_b)
    ln1_g, ln1_b, ln2_g, ln2_b = f32(ln1_g), f32(ln1_b), f32(ln2_g), f32(ln2_b)
    fc_w, fc_b = f32(fc_w), f32(fc_b)

    h0 = emb[x] * np.float32(SQRT_D)
    h0 = h0 + _pos_encoding_np()[:, None, :]
    h0t = np.transpose(h0, (0, 2, 1))          # [B, D, S]

    def aug(wT, bias):
        return np.concatenate([wT, bias[None, :]], axis=0).astype(np.float32)

    packw = np.zeros((37, 288), np.float32)
    for l in range(L):
        packw[0:D + 1, 70 * l:70 * l + D] = aug(
            in_proj_w[l][0:D].T * SCALE, in_proj_b[l][0:D] * SCALE)
        packw[0:D + 1, 70 * l + 32:70 * l + 32 + D] = aug(
            in_proj_w[l][D:2 * D].T, in_proj_b[l][D:2 * D])
        packw[0:D + 1, 70 * l + 64:70 * l + 64 + D] = aug(
            in_proj_w[l][2 * D:3 * D].T, in_proj_b[l][2 * D:3 * D])
        packw[0:D + 1, 140 + D * l:140 + D * (l + 1)] = aug(
            out_proj_w[l].T, out_proj_b[l])
        packw[0:D + 1, 150 + F * l:150 + F * (l + 1)] = aug(
            ff1_w[l].T, ff1_b[l])
        packw[0:F + 1, 190 + D * l:190 + D * (l + 1)] = aug(
            ff2_w[l].T, ff2_b[l])
        for i, (g, bb) in enumerate(((ln1_g[l], ln1_b[l]),
                                     (ln2_g[l], ln2_b[l]))):
            k = 2 * l + i
            packw[0, 200 + D * k:200 + D * (k + 1)] = g
            packw[32, 220 + D * k:220 + D * (k + 1)] = g
            packw[0:D, 278 + k] = bb
    packw[0:D, 240] = 1.0 / D          # c2 col 0: s1 = mean(x)
    packw[32:32 + D, 272] = 1.0 / D    # c2 col 32: s2 = mean(x^2)
    packw[0:D, 273:278] = np.eye(D, dtype=np.float32)
    packw[0, 283:288] = 1.0

    # multiplicative causal mask for the diagonal 128x128 block
    idx = np.arange(128)
    maskf = (idx[None, :] >= idx[:, None]).astype(np.float32)

    fcw = np.zeros((D + 1, VP), np.float32)
    fcw[0:D, 0:V] = fc_w.T
    fcw[D, 0:V] = fc_b
    fcw = fcw.astype(ml_dtypes.bfloat16)

    ctxa0 = np.ones((D + 1, S2), np.float32)
    f1a0 = np.ones((F + 1, S2), np.float32)
    vsb0 = np.zeros((128, QC, 37), np.float32)
    vsb0[:, :, 32:37] = 1.0
    hf0 = np.ones((D + 1, S2), ml_dtypes.bfloat16)
    rr0 = np.zeros((33, S), np.float32)

    shared = dict(packw=packw, mask=np.ascontiguousarray(maskf), fcw=fcw,
                  ctxa0=ctxa0, f1a0=f1a0, vsb0=vsb0, hf0=hf0, rr0=rr0)
    in_maps = []
    for c in range(NCORES):
        hh = np.ones((D + 1, S2), np.float32)
        for b in range(BPC):
            hh[0:D, b * S:(b + 1) * S] = h0t[c * BPC + b]
        in_maps.append(dict(h0=hh, **shared))
    return in_maps


def run(in_maps, trace=False, iters=1, **kw):
    nc = _get_program(iters)
    return run_bass_kernel_spmd(nc, in_maps, list(range(NCORES)),
                                trace=trace, **kw)


def unshard(res):
    """Per-core [128, BPC, QC, VP] fp16 -> [B, S, V] fp32."""
    outs = []
    for c in range(NCORES):
        a = np.asarray(res.results[c]["out"]).astype(np.float32)
        a = a.reshape(128, BPC, QC, VP)
        a = np.transpose(a, (1, 2, 0, 3)).reshape(BPC, S, VP)[:, :, 0:V]
        outs.append(a)
    return np.ascontiguousarray(np.concatenate(outs, axis=0))


def kernel(**inputs) -> np.ndarray:
    in_maps = host_inputs(**inputs)
    res = run(in_maps)
    return unshard(res)


if __name__ == "__main__":
    import reference
    ins = {k: np.asarray(v) for k, v in reference.setup_inputs().items()}
    got = kernel(**ins)
    exp = np.asarray(reference.reference(**reference.setup_inputs()))
    err = np.abs(got - exp)
    rel = err.max() / (np.abs(exp).max() + 1e-30)
    print("max abs err:", err.max(), "rel:", rel)
